# revision 29
# baseline (speedup 1.0000x reference)
"""Multi-head attention (B=2, S=2048, D=1024, H=16) on 8 Trainium2 cores.

Sharding: data-parallel over the 2 batches x tensor-parallel over 4 groups
of 4 heads.  Core c handles batch c//4 and heads [4*(c%4) : 4*(c%4)+4]
(columns [256*(c%4) : +256] of Wk/Wv, same rows of Wo).  Each core produces
a partial [S, D] output (its heads' contribution to o @ Wo); the host sums
the 4 partials per batch (and adds bo once).

Per-core dataflow (bf16 operands cast on HOST, fp32 PSUM accumulation):
  qT,kT,vT [D,S] bf16 (host-pre-transposed + cast) load over fast HWDGE.
  Projections produce QT,KT [128,2,S] (head-major rows) and V [sk,hd] with
  an extra ones column.  Attention per head in "scores-transposed" layout
  [sk_part, sq_free]: scoresT = KT_j^T @ QT; the causal diagonal adds a
  bf16 -480 lower-triangular tile into PSUM via an identity matmul; exp on
  ScalarE (scale folded in; no max subtraction - scores are O(6));
  UT[65, S] += Vaug_j^T @ expT accumulated in PSUM, row 64 = softmax
  denominators (from the ones column).  Normalization is region-wise
  (512 cols at a time, as soon as that region's last k-block lands):
  u copy out of PSUM (DVE) -> sums row SBUF-DMA to partition 0 -> in-place
  DVE reciprocal [1,512] -> gpsimd partition_broadcast -> gpsimd multiply
  into oT [d_part, sq] (keeps the DVE queue free for PSUM evacuations).
  Final: out = oT^T @ Wo per 128-row block, bf16 DMA to HBM (host sums
  partials in fp32).
"""

import os
from contextlib import ExitStack

import numpy as np

import concourse.bass as bass
import concourse.tile as tile
from concourse import bacc, bass_utils, mybir
from concourse.masks import make_identity

B, S, D, H = 2, 2048, 1024, 16
HD = D // H            # 64
NCORES = 8
HPC = 4                # heads per core
CW = HPC * HD          # 256 weight cols per core
NCH = 4                # sequence chunks of 512
MASKVAL = -480.0       # additive pre-scale causal mask value (exp -> ~e-60)
S_INV = float(1.0 / (np.sqrt(np.float32(HD)) + np.float32(1e-8)))

F32 = mybir.dt.float32
BF16 = mybir.dt.bfloat16


def _build(mode: str, bias_k: bool, bias_v: bool):
    """Build + compile the SPMD program.

    mode: 'causal' | 'none' | 'general'
    """
    nc = bacc.Bacc("TRN2", target_bir_lowering=False, debug=False,
                   num_devices=NCORES)

    qT_d = nc.dram_tensor("qT", [D, S], BF16, kind="ExternalInput").ap()
    kT_d = nc.dram_tensor("kT", [D, S], BF16, kind="ExternalInput").ap()
    vT_d = nc.dram_tensor("vT", [D, S], BF16, kind="ExternalInput").ap()
    # weights are host-prepacked partition-major so each load is one
    # contiguous 4KB-per-partition DMA (512B chunks are ~3x slower)
    wk_d = nc.dram_tensor("wk", [128, 8, CW], BF16, kind="ExternalInput").ap()
    wv_d = nc.dram_tensor("wv", [128, 8, CW], BF16, kind="ExternalInput").ap()
    wo_d = nc.dram_tensor("wo", [128, 2, D], BF16, kind="ExternalInput").ap()
    bk_d = nc.dram_tensor("bk", [1, CW], BF16, kind="ExternalInput").ap() if bias_k else None
    bv_d = nc.dram_tensor("bv", [1, CW], BF16, kind="ExternalInput").ap() if bias_v else None
    maskT_d = (nc.dram_tensor("maskT", [S, S], BF16, kind="ExternalInput").ap()
               if mode == "general" else None)
    ones1_d = (nc.dram_tensor("ones1", [1, 512], BF16, kind="ExternalInput").ap()
               if (bias_k or bias_v) else None)
    out_d = nc.dram_tensor("out", [S, D], BF16, kind="ExternalOutput").ap()

    with tile.TileContext(nc) as tc, ExitStack() as ctx:
        sb1 = ctx.enter_context(tc.tile_pool(name="persist", bufs=1))
        qt_pool = ctx.enter_context(tc.tile_pool(name="qt", bufs=NCH))
        kt_pool = ctx.enter_context(tc.tile_pool(name="kt", bufs=NCH))
        v_pool = ctx.enter_context(tc.tile_pool(name="v", bufs=NCH))
        stage_pool = ctx.enter_context(tc.tile_pool(name="stage", bufs=12))
        exp_pool = ctx.enter_context(tc.tile_pool(name="exp", bufs=6))
        u_pool = ctx.enter_context(tc.tile_pool(name="u", bufs=4))
        srt_pool = ctx.enter_context(tc.tile_pool(name="srt", bufs=4))
        rcb_pool = ctx.enter_context(tc.tile_pool(name="rcb", bufs=4))
        bc_pool = ctx.enter_context(tc.tile_pool(name="bc", bufs=5))
        ottmp_pool = ctx.enter_context(tc.tile_pool(name="ottmp", bufs=2))
        outsb_pool = ctx.enter_context(tc.tile_pool(name="outsb", bufs=4))
        sc_pool = ctx.enter_context(tc.tile_pool(name="sc", bufs=3, space="PSUM"))
        ut_pool = ctx.enter_context(tc.tile_pool(name="ut", bufs=1, space="PSUM"))
        if mode == "general":
            mask_pool = ctx.enter_context(tc.tile_pool(name="mask", bufs=3))

        # ---- constants / weights (all bf16, fast HWDGE loads) --------
        wk_sb = sb1.tile([128, 8, CW], BF16)
        nc.sync.dma_start(wk_sb[:], wk_d[:])
        wv_sb = sb1.tile([128, 8, CW], BF16)
        wo_sb = sb1.tile([128, 2, D], BF16)
        if bias_k:
            bk_sb = sb1.tile([1, CW], BF16)
            nc.sync.dma_start(bk_sb[:], bk_d[:])
        if bias_v:
            bv_sb = sb1.tile([1, CW], BF16)
            nc.sync.dma_start(bv_sb[:], bv_d[:])
        if bias_k or bias_v:
            ones_sb = sb1.tile([1, 512], BF16)
            nc.sync.dma_start(ones_sb[:], ones1_d[:])
        if mode == "general":
            ident = sb1.tile([128, 128], BF16)
            make_identity(nc, ident[:])
        if mode == "causal":
            # dmask[p, f] = MASKVAL where f < p (sq < sk), else 0.  fp32 so
            # DVE can add it straight into the scores PSUM (cheaper than an
            # identity matmul on the busy PE).
            dmask = sb1.tile([128, 128], F32)
            nc.gpsimd.memset(dmask[:], 0.0)
            nc.gpsimd.affine_select(
                out=dmask[:], in_=dmask[:],
                compare_op=mybir.AluOpType.is_ge,
                fill=MASKVAL, base=0,
                pattern=[[1, 128]], channel_multiplier=-1,
            )

        # V tiles: [128 sk, 4 blk, 4 head, 66] - col 64 is the ones column
        v_tiles = [v_pool.tile([128, 4, HPC, 66], BF16, tag="v", name=f"v{c}")
                   for c in range(NCH)]
        for c in range(NCH):
            nc.gpsimd.memset(v_tiles[c][:, :, :, 64:65], 1.0)
        qt_tiles = [qt_pool.tile([128, 2, 512], BF16, tag="qt", name=f"qt{c}")
                    for c in range(NCH)]
        kt_tiles = [kt_pool.tile([128, 2, 512], BF16, tag="kt", name=f"kt{c}")
                    for c in range(NCH)]
        oT_sb = sb1.tile([128, 2, S], BF16)

        def ps_copy(dst, src):
            nc.vector.tensor_copy(dst, src)

        # ---- phase 1: projections (helpers) --------------------------
        def emit_one_load(nm, td, c, hh, eng=None):
            sl = bass.ds(c * 512, 512)
            stg = stage_pool.tile([128, 4, 512], BF16, tag="stage",
                                  name=f"{nm}st{c}_{hh}")
            (eng or nc.sync).dma_start(
                stg[:], td.rearrange("(cc p) s -> p cc s", p=128)
                [:, bass.ds(4 * hh, 4), sl])
            return stg

        def emit_proj_loads(c):
            out = []
            order = ((("q", qT_d), ("k", kT_d), ("v", vT_d)) if c >= 2 else
                     (("k", kT_d), ("v", vT_d), ("q", qT_d)))
            for nm, td in order:
                out.append([emit_one_load(nm, td, c, hh) for hh in range(2)])
            return out

        def proj_mm_units(c, stages):
            """Yield once per PSUM accumulation group (small PE work unit)."""
            if c >= 2:
                (qst2, kst2, vst2) = stages
            else:
                (kst2, vst2, qst2) = stages
            class _Pair:
                def __init__(self, halves):
                    self.h = halves
                def __getitem__(self, key):
                    p, dc, rest = key[0], key[1], key[2:]
                    return self.h[dc // 4][(p, dc % 4) + rest]
            kst, vst, qst = _Pair(kst2), _Pair(vst2), _Pair(qst2)
            # KT / QT projections (transposed layout, 2 m-halves of 128)
            kq_order = (((qst, qt_tiles[c]), (kst, kt_tiles[c])) if c >= 2 else
                        ((kst, kt_tiles[c]), (qst, qt_tiles[c])))
            for ti, (st, dst) in enumerate(kq_order):
                for m in range(2):
                    ps = sc_pool.tile([128, 512], F32, tag="sc", name=f"psp{c}_{ti}_{m}")
                    first = True
                    if bias_k:
                        nc.tensor.matmul(ps[:], bk_sb[0:1, bass.ds(m * 128, 128)],
                                         ones_sb[0:1, :], start=True, stop=False)
                        first = False
                    for dc in range(8):
                        nc.tensor.matmul(
                            ps[:],
                            wk_sb[:, dc, bass.ds(m * 128, 128)],
                            st[:, dc, :],
                            start=first, stop=(dc == 7))
                        first = False
                    ps_copy(dst[:, m, :], ps[:])
                    yield
            # V projection (natural layout)
            for half in range(2):
                psv = sc_pool.tile([128, 512], F32, tag="sc", name=f"psv{c}_{half}")
                for loc in range(2):
                    blk = 2 * half + loc
                    reg = psv[:, bass.ds(loc * 256, 256)]
                    first = True
                    if bias_v:
                        nc.tensor.matmul(reg, ones_sb[0:1, 0:128], bv_sb[0:1, :],
                                         start=True, stop=False)
                        first = False
                    for dc in range(8):
                        nc.tensor.matmul(
                            reg,
                            vst[:, dc, bass.ds(blk * 128, 128)],
                            wv_sb[:, dc, :],
                            start=first, stop=(dc == 7))
                        first = False
                ps_copy(v_tiles[c][:, bass.ds(2 * half, 2), :, 0:64],
                        psv[:].rearrange("p (b h e) -> p b h e", b=2, h=HPC))
                yield

        # ---- phase 2: attention, one (head, sq-half) pass ------------
        full_grid = mode != "causal"

        # Normalization multiplies are deferred by one attention step so
        # the (in-order) DVE queue never sits waiting on the gpsimd
        # broadcast while a PE-critical PSUM evacuation is queued behind.
        deferred = []

        def flush_deferred():
            for f in deferred:
                f()
            deferred.clear()

        def attn_half(hl, half):
            m = hl // 2
            p0 = 64 * (hl % 2)
            base = 1024 * half
            regions = (2 * half, 2 * half + 1)
            ut = ut_pool.tile([128, 1024], F32, tag="ut", name=f"ut{hl}_{half}")

            if full_grid:
                steps = list(range(16))
                last_j = {r: 15 for r in regions}
            else:
                steps = list(range(8 * half + 8))
                last_j = {r: 4 * r + 3 for r in regions}

            win_ps = {}
            win_exp = {}

            def active(j):
                """absolute start col of k-block j's active window portion."""
                return base if full_grid else max(128 * j, base)

            def emit_scores(j):
                ps = sc_pool.tile([128, 1024], F32, tag="sc",
                                  name=f"sc{hl}_{half}_{j}")
                win_ps[j] = ps
                a0 = active(j)
                if mode == "general":
                    mt = mask_pool.tile([128, 1024], BF16, tag="mask",
                                        name=f"mt{hl}_{half}_{j}")
                    nc.sync.dma_start(
                        mt[:, a0 - base:],
                        maskT_d[bass.ds(128 * j, 128), bass.ds(a0, base + 1024 - a0)])
                lhsT = kt_tiles[j // 4][p0:p0 + 64, m, bass.ds(128 * (j % 4), 128)]
                for s in range(2):
                    lo, hi = base + 512 * s, base + 512 * s + 512
                    if hi <= a0:
                        continue
                    nlo = max(lo, a0)
                    n = hi - nlo
                    reg = ps[:, bass.ds(nlo - base, n)]
                    rhs = qt_tiles[nlo // 512][p0:p0 + 64, m, bass.ds(nlo % 512, n)]
                    mask_here = (mode == "general")
                    nc.tensor.matmul(reg, lhsT, rhs, start=True,
                                     stop=not mask_here)
                    if mask_here:
                        nc.tensor.matmul(reg, ident[:],
                                         mt[:, bass.ds(nlo - base, n)],
                                         start=False, stop=True)
                if (mode == "causal") and base <= 128 * j < base + 1024:
                    # causal diagonal: DVE-add the -480 lower-triangle into
                    # the scores PSUM (scores->exp edge has 2 steps slack)
                    dreg = ps[:, bass.ds(128 * j - base, 128)]
                    nc.vector.tensor_add(dreg, dreg, dmask[:])

            def emit_exp(j):
                ps = win_ps[j]
                off = active(j) - base
                et = exp_pool.tile([128, 1024], BF16, tag="exp",
                                   name=f"e{hl}_{half}_{j}")
                win_exp[j] = et
                nc.scalar.activation(et[:, off:1024], ps[:, off:1024],
                                     mybir.ActivationFunctionType.Exp, scale=S_INV)

            def emit_pv(j):
                et = win_exp.pop(j)
                win_ps.pop(j)
                a0 = active(j)
                for s in range(2):
                    lo, hi = base + 512 * s, base + 512 * s + 512
                    if hi <= a0:
                        continue
                    nlo = max(lo, a0)
                    r = nlo // 512
                    nc.tensor.matmul(
                        ut[0:65, bass.ds(nlo - base, hi - nlo)],
                        v_tiles[j // 4][:, j % 4, hl, 0:65],
                        et[:, bass.ds(nlo - base, hi - nlo)],
                        start=(j == 0), stop=(j == last_j[r]))

            if p0 == 0:
                dst = oT_sb[0:64, m, bass.ds(base, 1024)]
                ott = None
            else:
                ott = ottmp_pool.tile([64, 1024], BF16, tag="ottmp",
                                      name=f"ott{hl}_{half}")
                dst = ott[:, :]

            def emit_norm(r):
                """copy U+sums out of PSUM, then recip -> bcast -> multiply.

                The DMA reshape to [128,4] keeps the reciprocal's per-lane
                free size at 4 (a [1,512] recip costs ~3.3us).  gpsimd runs
                ONLY partition_broadcast in steady state - mixing in other
                gpsimd op types causes ~7us DSP LIBRARY_RELOADs.  The DVE
                normalize-multiply is deferred one step (see above)."""
                u = u_pool.tile([65, 512], F32, tag="u", name=f"u{hl}_{r}")
                nc.vector.tensor_copy(u[:], ut[0:65, bass.ds(512 * r - base, 512)])
                srt = srt_pool.tile([128, 4], F32, tag="srt", name=f"srt{hl}_{r}")
                nc.sync.dma_start(srt[:], u[64:65, :])
                nc.vector.reciprocal(srt[:], srt[:])
                rcb = rcb_pool.tile([1, 512], F32, tag="rcb", name=f"rcb{hl}_{r}")
                nc.sync.dma_start(rcb[0:1, :], srt[:])
                bc = bc_pool.tile([64, 512], F32, tag="bc", name=f"bc{hl}_{r}")
                nc.gpsimd.partition_broadcast(bc[:], rcb[:], channels=64)

                def _mul():
                    nc.vector.tensor_mul(
                        dst[:, bass.ds(512 * r - base, 512)],
                        u[0:64, :],
                        bc[:, :])
                    if p0:
                        nc.sync.dma_start(
                            oT_sb[64:128, m, bass.ds(512 * r, 512)],
                            ott[:, bass.ds(512 * r - base, 512)])
                deferred.append(_mul)

            LOOKAHEAD = 2
            for i in range(min(LOOKAHEAD, len(steps))):
                emit_scores(steps[i])
            for i, j in enumerate(steps):
                flush_deferred()
                if i + LOOKAHEAD < len(steps):
                    emit_scores(steps[i + LOOKAHEAD])
                emit_exp(j)
                emit_pv(j)
                for r in regions:
                    if j == last_j[r]:
                        emit_norm(r)
                yield

        def emit_final(sb):
            # output stores go out on gpsimd/SWDGE so the sync queue stays
            # clear for the latency-critical softmax-norm reshape DMAs
            ob = outsb_pool.tile([128, D], BF16, tag="outsb", name=f"ob{sb}")
            for nh in range(2):
                ps = sc_pool.tile([128, 512], F32, tag="sc", name=f"pso{sb}_{nh}")
                for mm_ in range(2):
                    nc.tensor.matmul(
                        ps[:],
                        oT_sb[:, mm_, bass.ds(sb * 128, 128)],
                        wo_sb[:, mm_, bass.ds(nh * 512, 512)],
                        start=(mm_ == 0), stop=(mm_ == 1))
                ps_copy(ob[:, bass.ds(nh * 512, 512)], ps[:])
                nc.gpsimd.dma_start(
                    out_d[bass.ds(sb * 128, 128), bass.ds(nh * 512, 512)],
                    ob[:, bass.ds(nh * 512, 512)])

        def drain(gen):
            for _ in gen:
                pass

        def weave(step_gen, unit_gen, steps_per_unit):
            """Emit attention steps, inserting one PE-heavy unit every N."""
            i = 0
            for _ in step_gen:
                i += 1
                if i % steps_per_unit == 0:
                    next(unit_gen, None)
            for _ in unit_gen:
                pass

        def chain(*gens):
            for g in gens:
                yield from g

        # ---- orchestration: overlap proj DMA with attention ----------
        # load order matches first-consumer order: wk -> k0 -> q0 (KQ proj
        # units) -> wv -> v0 (V proj unit) -> chunk 1 -> wo
        kst0 = [emit_one_load("k", kT_d, 0, hh) for hh in range(2)]
        qst0 = [emit_one_load("q", qT_d, 0, hh) for hh in range(2)]
        nc.sync.dma_start(wv_sb[:], wv_d[:])
        vst0 = [emit_one_load("v", vT_d, 0, hh) for hh in range(2)]
        st0 = [kst0, vst0, qst0]
        st1 = emit_proj_loads(1)
        nc.sync.dma_start(wo_sb[:], wo_d[:])
        drain(proj_mm_units(0, st0))
        st2 = emit_proj_loads(2)
        st3 = emit_proj_loads(3)
        drain(proj_mm_units(1, st1))
        if full_grid:
            # every k-block needs every chunk: project everything first
            drain(proj_mm_units(2, st2))
            drain(proj_mm_units(3, st3))
            for hl in (1, 3, 0, 2):
                drain(attn_half(hl, 0))
        else:
            half0s = chain(*[attn_half(hl, 0) for hl in (1, 3, 0, 2)])
            proj23 = chain(proj_mm_units(2, st2), proj_mm_units(3, st3))
            # small un-woven prefix: let the c2/c3 stage DMAs land so an
            # early proj unit can't head-of-line-block the in-order PE.
            for _ in range(6):
                next(half0s, None)
            weave(half0s, proj23, 1)

        def final_units(lo, hi):
            for sb in range(lo, hi):
                emit_final(sb)
                yield

        half1s_012 = chain(*[attn_half(hl, 1) for hl in (1, 3, 0)])
        weave(half1s_012, final_units(0, 8), 5)
        f811 = final_units(8, 12)
        thr = 12 if mode == "causal" else 16  # step after region-2 norm
        for i, _ in enumerate(attn_half(2, 1)):
            if i >= thr:
                next(f811, None)
        for _ in f811:
            pass
        flush_deferred()
        for sb in range(12, 16):
            emit_final(sb)


    nc.compile()
    return nc


_CACHE = {}


def _get_nc(mode, bias_k, bias_v):
    key = (mode, bias_k, bias_v)
    if key not in _CACHE:
        _CACHE[key] = _build(mode, bias_k, bias_v)
    return _CACHE[key]


def make_in_maps(q, k, v, mask, Wk, bk, Wv, bv, Wo, bo):
    """Host-side sharding. Returns (mode, bias flags, in_maps)."""
    import ml_dtypes

    BF = ml_dtypes.bfloat16

    q = np.asarray(q, dtype=np.float32)
    k = np.asarray(k, dtype=np.float32)
    v = np.asarray(v, dtype=np.float32)
    Wk = np.asarray(Wk, dtype=np.float32).astype(BF)
    Wv = np.asarray(Wv, dtype=np.float32).astype(BF)
    Wo = np.asarray(Wo, dtype=np.float32).astype(BF)
    bk = np.asarray(bk, dtype=np.float32).reshape(-1)
    bv = np.asarray(bv, dtype=np.float32).reshape(-1)
    bo = np.asarray(bo, dtype=np.float32).reshape(-1)
    mask2d = np.asarray(mask, dtype=np.float32).reshape(S, S)

    if not mask2d.any():
        mode = "none"
    elif np.array_equal(mask2d, np.triu(np.ones((S, S), np.float32), 1)):
        mode = "causal"
    else:
        mode = "general"
    bias_k, bias_v, bias_o = bool(bk.any()), bool(bv.any()), bool(bo.any())

    qT = [np.ascontiguousarray(q[b].T).astype(BF) for b in range(B)]
    kT = [np.ascontiguousarray(k[b].T).astype(BF) for b in range(B)]
    vT = [np.ascontiguousarray(v[b].T).astype(BF) for b in range(B)]
    if mode == "general":
        # pre-scale so adding before the fused exp scale matches the
        # reference's post-scale add:  (raw + m)*S_INV == raw*S_INV + mask*(-1e9)
        maskT = np.ascontiguousarray(
            (mask2d.T * np.float32(-1e9 / S_INV)).astype(BF))

    def pack_pmajor(w, groups):
        """[groups*128, n] -> [128, groups, n] partition-major prepack."""
        n = w.shape[1]
        return np.ascontiguousarray(
            w.reshape(groups, 128, n).transpose(1, 0, 2))

    in_maps = []
    for core in range(NCORES):
        b, g = divmod(core, HPC)
        cs = slice(CW * g, CW * (g + 1))
        im = {
            "qT": qT[b], "kT": kT[b], "vT": vT[b],
            "wk": pack_pmajor(np.ascontiguousarray(Wk[:, cs]), 8),
            "wv": pack_pmajor(np.ascontiguousarray(Wv[:, cs]), 8),
            "wo": pack_pmajor(np.ascontiguousarray(Wo[cs, :]), 2),
        }
        if bias_k or bias_v:
            im["ones1"] = np.ones((1, 512), dtype=BF)
        if bias_k:
            im["bk"] = np.ascontiguousarray(bk[cs].astype(BF)).reshape(1, CW)
        if bias_v:
            im["bv"] = np.ascontiguousarray(bv[cs].astype(BF)).reshape(1, CW)
        if mode == "general":
            im["maskT"] = maskT
        in_maps.append(im)
    return mode, (bias_k, bias_v, bias_o), in_maps


def assemble(results, bo=None):
    """Sum per-core partial outputs into the full [B, S, D] output."""
    full = np.zeros((B, S, D), dtype=np.float32)
    for b in range(B):
        acc = results[4 * b]["out"].astype(np.float32)
        for c in range(4 * b + 1, 4 * b + 4):
            acc = acc + results[c]["out"].astype(np.float32)
        if bo is not None:
            acc = acc + bo
        full[b] = acc
    return full


def kernel(q, k, v, mask, Wk, bk, Wv, bv, Wo, bo):
    mode, (bias_k, bias_v, bias_o), in_maps = make_in_maps(
        q, k, v, mask, Wk, bk, Wv, bv, Wo, bo)
    nc = _get_nc(mode, bias_k, bias_v)
    res = bass_utils.run_bass_kernel_spmd(nc, in_maps, core_ids=list(range(NCORES)))
    bo_arr = np.asarray(bo, dtype=np.float32).reshape(-1) if bias_o else None
    return assemble(res.results, bo_arr)


# revision 32
# speedup vs baseline: 1.0570x; 1.0570x over previous
"""Multi-head attention (B=2, S=2048, D=1024, H=16) on 8 Trainium2 cores.

Sharding: data-parallel over the 2 batches x tensor-parallel over 4 groups
of 4 heads.  Core c handles batch c//4 and heads [4*(c%4) : 4*(c%4)+4]
(columns [256*(c%4) : +256] of Wk/Wv, same rows of Wo).  Each core produces
a partial [S, D] output (its heads' contribution to o @ Wo); the host sums
the 4 partials per batch (and adds bo once).

Per-core dataflow (bf16 operands cast on HOST, fp32 PSUM accumulation):
  qT,kT,vT [D,S] bf16 (host-pre-transposed + cast) load over fast HWDGE.
  Projections produce QT,KT [128,2,S] (head-major rows) and V [sk,hd] with
  an extra ones column.  Attention per head in "scores-transposed" layout
  [sk_part, sq_free]: scoresT = KT_j^T @ QT; the causal diagonal adds a
  bf16 -480 lower-triangular tile into PSUM via an identity matmul; exp on
  ScalarE (scale folded in; no max subtraction - scores are O(6));
  UT[65, S] += Vaug_j^T @ expT accumulated in PSUM, row 64 = softmax
  denominators (from the ones column).  Normalization is region-wise
  (512 cols at a time, as soon as that region's last k-block lands):
  u copy out of PSUM (DVE) -> sums row SBUF-DMA to partition 0 -> in-place
  DVE reciprocal [1,512] -> gpsimd partition_broadcast -> gpsimd multiply
  into oT [d_part, sq] (keeps the DVE queue free for PSUM evacuations).
  Final: out = oT^T @ Wo per 128-row block, bf16 DMA to HBM (host sums
  partials in fp32).
"""

import os
from contextlib import ExitStack

import numpy as np

import concourse.bass as bass
import concourse.tile as tile
from concourse import bacc, bass_utils, mybir
from concourse.masks import make_identity

B, S, D, H = 2, 2048, 1024, 16
HD = D // H            # 64
NCORES = 8
HPC = 4                # heads per core
CW = HPC * HD          # 256 weight cols per core
NCH = 4                # sequence chunks of 512
MASKVAL = -480.0       # additive pre-scale causal mask value (exp -> ~e-60)
S_INV = float(1.0 / (np.sqrt(np.float32(HD)) + np.float32(1e-8)))

F32 = mybir.dt.float32
BF16 = mybir.dt.bfloat16


def _build(mode: str, bias_k: bool, bias_v: bool):
    """Build + compile the SPMD program.

    mode: 'causal' | 'none' | 'general'
    """
    nc = bacc.Bacc("TRN2", target_bir_lowering=False, debug=False,
                   num_devices=NCORES)

    qT_d = nc.dram_tensor("qT", [D, S], BF16, kind="ExternalInput").ap()
    kT_d = nc.dram_tensor("kT", [D, S], BF16, kind="ExternalInput").ap()
    vT_d = nc.dram_tensor("vT", [D, S], BF16, kind="ExternalInput").ap()
    # weights are host-prepacked partition-major so each load is one
    # contiguous 4KB-per-partition DMA (512B chunks are ~3x slower)
    wk_d = nc.dram_tensor("wk", [128, 8, CW], BF16, kind="ExternalInput").ap()
    wv_d = nc.dram_tensor("wv", [128, 8, CW], BF16, kind="ExternalInput").ap()
    wo_d = nc.dram_tensor("wo", [128, 2, D], BF16, kind="ExternalInput").ap()
    bk_d = nc.dram_tensor("bk", [1, CW], BF16, kind="ExternalInput").ap() if bias_k else None
    bv_d = nc.dram_tensor("bv", [1, CW], BF16, kind="ExternalInput").ap() if bias_v else None
    maskT_d = (nc.dram_tensor("maskT", [S, S], BF16, kind="ExternalInput").ap()
               if mode == "general" else None)
    ones1_d = (nc.dram_tensor("ones1", [1, 512], BF16, kind="ExternalInput").ap()
               if (bias_k or bias_v) else None)
    out_d = nc.dram_tensor("out", [S, D], BF16, kind="ExternalOutput").ap()

    with tile.TileContext(nc) as tc, ExitStack() as ctx:
        sb1 = ctx.enter_context(tc.tile_pool(name="persist", bufs=1))
        qt_pool = ctx.enter_context(tc.tile_pool(name="qt", bufs=NCH))
        kt_pool = ctx.enter_context(tc.tile_pool(name="kt", bufs=NCH))
        v_pool = ctx.enter_context(tc.tile_pool(name="v", bufs=NCH))
        stage_pool = ctx.enter_context(tc.tile_pool(name="stage", bufs=12))
        exp_pool = ctx.enter_context(tc.tile_pool(name="exp", bufs=6))
        u_pool = ctx.enter_context(tc.tile_pool(name="u", bufs=4))
        srt_pool = ctx.enter_context(tc.tile_pool(name="srt", bufs=4))
        rcb_pool = ctx.enter_context(tc.tile_pool(name="rcb", bufs=4))
        bc_pool = ctx.enter_context(tc.tile_pool(name="bc", bufs=5))
        ottmp_pool = ctx.enter_context(tc.tile_pool(name="ottmp", bufs=2))
        outsb_pool = ctx.enter_context(tc.tile_pool(name="outsb", bufs=4))
        sc_pool = ctx.enter_context(tc.tile_pool(name="sc", bufs=3, space="PSUM"))
        ut_pool = ctx.enter_context(tc.tile_pool(name="ut", bufs=1, space="PSUM"))
        if mode == "general":
            mask_pool = ctx.enter_context(tc.tile_pool(name="mask", bufs=3))

        # ---- constants / weights (all bf16, fast HWDGE loads) --------
        wk_sb = sb1.tile([128, 8, CW], BF16)
        nc.sync.dma_start(wk_sb[:], wk_d[:])
        wv_sb = sb1.tile([128, 8, CW], BF16)
        wo_sb = sb1.tile([128, 2, D], BF16)
        if bias_k:
            bk_sb = sb1.tile([1, CW], BF16)
            nc.sync.dma_start(bk_sb[:], bk_d[:])
        if bias_v:
            bv_sb = sb1.tile([1, CW], BF16)
            nc.sync.dma_start(bv_sb[:], bv_d[:])
        if bias_k or bias_v:
            ones_sb = sb1.tile([1, 512], BF16)
            nc.sync.dma_start(ones_sb[:], ones1_d[:])
        if mode != "none":
            ident = sb1.tile([128, 128], BF16)
            make_identity(nc, ident[:])
        if mode == "causal":
            # dmask[p, f] = MASKVAL where f < p (sq < sk), else 0
            dmask = sb1.tile([128, 128], BF16)
            nc.gpsimd.memset(dmask[:], 0.0)
            nc.gpsimd.affine_select(
                out=dmask[:], in_=dmask[:],
                compare_op=mybir.AluOpType.is_ge,
                fill=MASKVAL, base=0,
                pattern=[[1, 128]], channel_multiplier=-1,
            )

        # V tiles: [128 sk, 4 blk, 4 head, 66] - col 64 is the ones column
        v_tiles = [v_pool.tile([128, 4, HPC, 66], BF16, tag="v", name=f"v{c}")
                   for c in range(NCH)]
        for c in range(NCH):
            nc.gpsimd.memset(v_tiles[c][:, :, :, 64:65], 1.0)
        qt_tiles = [qt_pool.tile([128, 2, 512], BF16, tag="qt", name=f"qt{c}")
                    for c in range(NCH)]
        kt_tiles = [kt_pool.tile([128, 2, 512], BF16, tag="kt", name=f"kt{c}")
                    for c in range(NCH)]
        oT_sb = sb1.tile([128, 2, S], BF16)

        def ps_copy(dst, src):
            nc.vector.tensor_copy(dst, src)

        # ---- phase 1: projections (helpers) --------------------------
        def emit_one_load(nm, td, c, hh, eng=None):
            sl = bass.ds(c * 512, 512)
            stg = stage_pool.tile([128, 4, 512], BF16, tag="stage",
                                  name=f"{nm}st{c}_{hh}")
            (eng or nc.sync).dma_start(
                stg[:], td.rearrange("(cc p) s -> p cc s", p=128)
                [:, bass.ds(4 * hh, 4), sl])
            return stg

        def emit_proj_loads(c):
            # load order = first-consumer order: c<2 units run K,Q then V
            order = ((("q", qT_d), ("k", kT_d), ("v", vT_d)) if c >= 2 else
                     (("k", kT_d), ("q", qT_d), ("v", vT_d)))
            ld = {nm: [emit_one_load(nm, td, c, hh) for hh in range(2)]
                  for nm, td in order}
            return ([ld["q"], ld["k"], ld["v"]] if c >= 2 else
                    [ld["k"], ld["v"], ld["q"]])

        def proj_mm_units(c, stages):
            """Yield once per PSUM accumulation group (small PE work unit)."""
            if c >= 2:
                (qst2, kst2, vst2) = stages
            else:
                (kst2, vst2, qst2) = stages
            class _Pair:
                def __init__(self, halves):
                    self.h = halves
                def __getitem__(self, key):
                    p, dc, rest = key[0], key[1], key[2:]
                    return self.h[dc // 4][(p, dc % 4) + rest]
            kst, vst, qst = _Pair(kst2), _Pair(vst2), _Pair(qst2)
            # KT / QT projections (transposed layout, 2 m-halves of 128)
            kq_order = (((qst, qt_tiles[c]), (kst, kt_tiles[c])) if c >= 2 else
                        ((kst, kt_tiles[c]), (qst, qt_tiles[c])))
            for ti, (st, dst) in enumerate(kq_order):
                for m in range(2):
                    ps = sc_pool.tile([128, 512], F32, tag="sc", name=f"psp{c}_{ti}_{m}")
                    first = True
                    if bias_k:
                        nc.tensor.matmul(ps[:], bk_sb[0:1, bass.ds(m * 128, 128)],
                                         ones_sb[0:1, :], start=True, stop=False)
                        first = False
                    for dc in range(8):
                        nc.tensor.matmul(
                            ps[:],
                            wk_sb[:, dc, bass.ds(m * 128, 128)],
                            st[:, dc, :],
                            start=first, stop=(dc == 7))
                        first = False
                    ps_copy(dst[:, m, :], ps[:])
                    yield
            # V projection (natural layout)
            for half in range(2):
                psv = sc_pool.tile([128, 512], F32, tag="sc", name=f"psv{c}_{half}")
                for loc in range(2):
                    blk = 2 * half + loc
                    reg = psv[:, bass.ds(loc * 256, 256)]
                    first = True
                    if bias_v:
                        nc.tensor.matmul(reg, ones_sb[0:1, 0:128], bv_sb[0:1, :],
                                         start=True, stop=False)
                        first = False
                    for dc in range(8):
                        nc.tensor.matmul(
                            reg,
                            vst[:, dc, bass.ds(blk * 128, 128)],
                            wv_sb[:, dc, :],
                            start=first, stop=(dc == 7))
                        first = False
                ps_copy(v_tiles[c][:, bass.ds(2 * half, 2), :, 0:64],
                        psv[:].rearrange("p (b h e) -> p b h e", b=2, h=HPC))
                yield

        # ---- phase 2: attention, one (head, sq-half) pass ------------
        full_grid = mode != "causal"

        # Normalization multiplies are deferred by one attention step so
        # the (in-order) DVE queue never sits waiting on the gpsimd
        # broadcast while a PE-critical PSUM evacuation is queued behind.
        deferred = []

        def flush_deferred():
            for f in deferred:
                f()
            deferred.clear()

        def attn_half(hl, half):
            m = hl // 2
            p0 = 64 * (hl % 2)
            base = 1024 * half
            regions = (2 * half, 2 * half + 1)
            ut = ut_pool.tile([128, 1024], F32, tag="ut", name=f"ut{hl}_{half}")

            if full_grid:
                steps = list(range(16))
                last_j = {r: 15 for r in regions}
            else:
                steps = list(range(8 * half + 8))
                last_j = {r: 4 * r + 3 for r in regions}

            win_ps = {}
            win_exp = {}

            def active(j):
                """absolute start col of k-block j's active window portion."""
                return base if full_grid else max(128 * j, base)

            def emit_scores(j):
                ps = sc_pool.tile([128, 1024], F32, tag="sc",
                                  name=f"sc{hl}_{half}_{j}")
                win_ps[j] = ps
                a0 = active(j)
                if mode == "general":
                    mt = mask_pool.tile([128, 1024], BF16, tag="mask",
                                        name=f"mt{hl}_{half}_{j}")
                    nc.sync.dma_start(
                        mt[:, a0 - base:],
                        maskT_d[bass.ds(128 * j, 128), bass.ds(a0, base + 1024 - a0)])
                lhsT = kt_tiles[j // 4][p0:p0 + 64, m, bass.ds(128 * (j % 4), 128)]
                for s in range(2):
                    lo, hi = base + 512 * s, base + 512 * s + 512
                    if hi <= a0:
                        continue
                    nlo = max(lo, a0)
                    n = hi - nlo
                    reg = ps[:, bass.ds(nlo - base, n)]
                    rhs = qt_tiles[nlo // 512][p0:p0 + 64, m, bass.ds(nlo % 512, n)]
                    diag_here = (mode == "causal") and lo <= 128 * j < hi
                    mask_here = (mode == "general")
                    nc.tensor.matmul(reg, lhsT, rhs, start=True,
                                     stop=not (diag_here or mask_here))
                    if diag_here:
                        nc.tensor.matmul(ps[:, bass.ds(128 * j - base, 128)],
                                         ident[:], dmask[:], start=False, stop=True)
                    elif mask_here:
                        nc.tensor.matmul(reg, ident[:],
                                         mt[:, bass.ds(nlo - base, n)],
                                         start=False, stop=True)

            def emit_exp(j):
                ps = win_ps[j]
                off = active(j) - base
                et = exp_pool.tile([128, 1024], BF16, tag="exp",
                                   name=f"e{hl}_{half}_{j}")
                win_exp[j] = et
                nc.scalar.activation(et[:, off:1024], ps[:, off:1024],
                                     mybir.ActivationFunctionType.Exp, scale=S_INV)

            def emit_pv(j):
                et = win_exp.pop(j)
                win_ps.pop(j)
                a0 = active(j)
                for s in range(2):
                    lo, hi = base + 512 * s, base + 512 * s + 512
                    if hi <= a0:
                        continue
                    nlo = max(lo, a0)
                    r = nlo // 512
                    nc.tensor.matmul(
                        ut[0:65, bass.ds(nlo - base, hi - nlo)],
                        v_tiles[j // 4][:, j % 4, hl, 0:65],
                        et[:, bass.ds(nlo - base, hi - nlo)],
                        start=(j == 0), stop=(j == last_j[r]))

            if p0 == 0:
                dst = oT_sb[0:64, m, bass.ds(base, 1024)]
                ott = None
            else:
                ott = ottmp_pool.tile([64, 1024], BF16, tag="ottmp",
                                      name=f"ott{hl}_{half}")
                dst = ott[:, :]

            def emit_norm(r):
                """copy U+sums out of PSUM, then recip -> bcast -> multiply.

                The DMA reshape to [128,4] keeps the reciprocal's per-lane
                free size at 4 (a [1,512] recip costs ~3.3us).  gpsimd runs
                ONLY partition_broadcast in steady state - mixing in other
                gpsimd op types causes ~7us DSP LIBRARY_RELOADs.  The DVE
                normalize-multiply is deferred one step (see above)."""
                u = u_pool.tile([65, 512], F32, tag="u", name=f"u{hl}_{r}")
                nc.vector.tensor_copy(u[:], ut[0:65, bass.ds(512 * r - base, 512)])
                srt = srt_pool.tile([128, 4], F32, tag="srt", name=f"srt{hl}_{r}")
                nc.sync.dma_start(srt[:], u[64:65, :])
                nc.vector.reciprocal(srt[:], srt[:])
                rcb = rcb_pool.tile([1, 512], F32, tag="rcb", name=f"rcb{hl}_{r}")
                nc.sync.dma_start(rcb[0:1, :], srt[:])
                bc = bc_pool.tile([64, 512], F32, tag="bc", name=f"bc{hl}_{r}")
                nc.gpsimd.partition_broadcast(bc[:], rcb[:], channels=64)

                def _mul():
                    nc.vector.tensor_mul(
                        dst[:, bass.ds(512 * r - base, 512)],
                        u[0:64, :],
                        bc[:, :])
                    if p0:
                        nc.sync.dma_start(
                            oT_sb[64:128, m, bass.ds(512 * r, 512)],
                            ott[:, bass.ds(512 * r - base, 512)])
                deferred.append(_mul)

            LOOKAHEAD = 2
            for i in range(min(LOOKAHEAD, len(steps))):
                emit_scores(steps[i])
            for i, j in enumerate(steps):
                flush_deferred()
                if i + LOOKAHEAD < len(steps):
                    emit_scores(steps[i + LOOKAHEAD])
                emit_exp(j)
                emit_pv(j)
                for r in regions:
                    if j == last_j[r]:
                        emit_norm(r)
                yield

        def emit_final(sb):
            # output stores go out on gpsimd/SWDGE so the sync queue stays
            # clear for the latency-critical softmax-norm reshape DMAs
            ob = outsb_pool.tile([128, D], BF16, tag="outsb", name=f"ob{sb}")
            for nh in range(2):
                ps = sc_pool.tile([128, 512], F32, tag="sc", name=f"pso{sb}_{nh}")
                for mm_ in range(2):
                    nc.tensor.matmul(
                        ps[:],
                        oT_sb[:, mm_, bass.ds(sb * 128, 128)],
                        wo_sb[:, mm_, bass.ds(nh * 512, 512)],
                        start=(mm_ == 0), stop=(mm_ == 1))
                ps_copy(ob[:, bass.ds(nh * 512, 512)], ps[:])
                nc.gpsimd.dma_start(
                    out_d[bass.ds(sb * 128, 128), bass.ds(nh * 512, 512)],
                    ob[:, bass.ds(nh * 512, 512)])

        def drain(gen):
            for _ in gen:
                pass

        def weave(step_gen, unit_gen, steps_per_unit):
            """Emit attention steps, inserting one PE-heavy unit every N."""
            i = 0
            for _ in step_gen:
                i += 1
                if i % steps_per_unit == 0:
                    next(unit_gen, None)
            for _ in unit_gen:
                pass

        def chain(*gens):
            for g in gens:
                yield from g

        # ---- orchestration: overlap proj DMA with attention ----------
        # load order matches first-consumer order: wk -> k0 -> q0 (KQ proj
        # units) -> wv -> v0 (V proj unit) -> chunk 1 -> wo
        kst0 = [emit_one_load("k", kT_d, 0, hh) for hh in range(2)]
        qst0 = [emit_one_load("q", qT_d, 0, hh) for hh in range(2)]
        nc.sync.dma_start(wv_sb[:], wv_d[:])
        vst0 = [emit_one_load("v", vT_d, 0, hh) for hh in range(2)]
        st0 = [kst0, vst0, qst0]
        st1 = emit_proj_loads(1)
        nc.sync.dma_start(wo_sb[:], wo_d[:])
        drain(proj_mm_units(0, st0))
        st2 = emit_proj_loads(2)
        st3 = emit_proj_loads(3)
        drain(proj_mm_units(1, st1))
        if full_grid:
            # every k-block needs every chunk: project everything first
            drain(proj_mm_units(2, st2))
            drain(proj_mm_units(3, st3))
            for hl in (1, 3, 0, 2):
                drain(attn_half(hl, 0))
        else:
            half0s = chain(*[attn_half(hl, 0) for hl in (1, 3, 0, 2)])
            proj23 = chain(proj_mm_units(2, st2), proj_mm_units(3, st3))
            # small un-woven prefix: let the c2/c3 stage DMAs land so an
            # early proj unit can't head-of-line-block the in-order PE.
            for _ in range(6):
                next(half0s, None)
            weave(half0s, proj23, 1)

        def final_units(lo, hi):
            for sb in range(lo, hi):
                emit_final(sb)
                yield

        half1s_012 = chain(*[attn_half(hl, 1) for hl in (1, 3, 0)])
        weave(half1s_012, final_units(0, 8), 5)
        f811 = final_units(8, 12)
        thr = 12 if mode == "causal" else 16  # step after region-2 norm
        for i, _ in enumerate(attn_half(2, 1)):
            if i >= thr:
                next(f811, None)
        for _ in f811:
            pass
        # endgame: the last pass's region-3 norm chain has ~7us of
        # cross-engine latency.  The m=0 halves of the last finals only
        # need heads 0/1 (normalized long ago), so open those accumulation
        # groups now and let the PE chew on them while the chain drains.
        def open_final(sb):
            ps = sc_pool.tile([128, 1024], F32, tag="sc", name=f"psoX{sb}")
            for nh in range(2):
                nc.tensor.matmul(
                    ps[:, bass.ds(nh * 512, 512)],
                    oT_sb[:, 0, bass.ds(sb * 128, 128)],
                    wo_sb[:, 0, bass.ds(nh * 512, 512)],
                    start=True, stop=False)
            return ps

        def close_final(sb, ps):
            ob = outsb_pool.tile([128, D], BF16, tag="outsb", name=f"ob{sb}")
            for nh in range(2):
                nc.tensor.matmul(
                    ps[:, bass.ds(nh * 512, 512)],
                    oT_sb[:, 1, bass.ds(sb * 128, 128)],
                    wo_sb[:, 1, bass.ds(nh * 512, 512)],
                    start=False, stop=True)
                ps_copy(ob[:, bass.ds(nh * 512, 512)],
                        ps[:, bass.ds(nh * 512, 512)])
                nc.gpsimd.dma_start(
                    out_d[bass.ds(sb * 128, 128), bass.ds(nh * 512, 512)],
                    ob[:, bass.ds(nh * 512, 512)])

        ps12 = open_final(12)
        ps13 = open_final(13)
        flush_deferred()
        close_final(12, ps12)
        close_final(13, ps13)
        for sb in range(14, 16):
            emit_final(sb)


    nc.compile()
    return nc


_CACHE = {}


def _get_nc(mode, bias_k, bias_v):
    key = (mode, bias_k, bias_v)
    if key not in _CACHE:
        _CACHE[key] = _build(mode, bias_k, bias_v)
    return _CACHE[key]


def make_in_maps(q, k, v, mask, Wk, bk, Wv, bv, Wo, bo):
    """Host-side sharding. Returns (mode, bias flags, in_maps)."""
    import ml_dtypes

    BF = ml_dtypes.bfloat16

    q = np.asarray(q, dtype=np.float32)
    k = np.asarray(k, dtype=np.float32)
    v = np.asarray(v, dtype=np.float32)
    Wk = np.asarray(Wk, dtype=np.float32).astype(BF)
    Wv = np.asarray(Wv, dtype=np.float32).astype(BF)
    Wo = np.asarray(Wo, dtype=np.float32).astype(BF)
    bk = np.asarray(bk, dtype=np.float32).reshape(-1)
    bv = np.asarray(bv, dtype=np.float32).reshape(-1)
    bo = np.asarray(bo, dtype=np.float32).reshape(-1)
    mask2d = np.asarray(mask, dtype=np.float32).reshape(S, S)

    if not mask2d.any():
        mode = "none"
    elif np.array_equal(mask2d, np.triu(np.ones((S, S), np.float32), 1)):
        mode = "causal"
    else:
        mode = "general"
    bias_k, bias_v, bias_o = bool(bk.any()), bool(bv.any()), bool(bo.any())

    qT = [np.ascontiguousarray(q[b].T).astype(BF) for b in range(B)]
    kT = [np.ascontiguousarray(k[b].T).astype(BF) for b in range(B)]
    vT = [np.ascontiguousarray(v[b].T).astype(BF) for b in range(B)]
    if mode == "general":
        # pre-scale so adding before the fused exp scale matches the
        # reference's post-scale add:  (raw + m)*S_INV == raw*S_INV + mask*(-1e9)
        maskT = np.ascontiguousarray(
            (mask2d.T * np.float32(-1e9 / S_INV)).astype(BF))

    def pack_pmajor(w, groups):
        """[groups*128, n] -> [128, groups, n] partition-major prepack."""
        n = w.shape[1]
        return np.ascontiguousarray(
            w.reshape(groups, 128, n).transpose(1, 0, 2))

    in_maps = []
    for core in range(NCORES):
        b, g = divmod(core, HPC)
        cs = slice(CW * g, CW * (g + 1))
        im = {
            "qT": qT[b], "kT": kT[b], "vT": vT[b],
            "wk": pack_pmajor(np.ascontiguousarray(Wk[:, cs]), 8),
            "wv": pack_pmajor(np.ascontiguousarray(Wv[:, cs]), 8),
            "wo": pack_pmajor(np.ascontiguousarray(Wo[cs, :]), 2),
        }
        if bias_k or bias_v:
            im["ones1"] = np.ones((1, 512), dtype=BF)
        if bias_k:
            im["bk"] = np.ascontiguousarray(bk[cs].astype(BF)).reshape(1, CW)
        if bias_v:
            im["bv"] = np.ascontiguousarray(bv[cs].astype(BF)).reshape(1, CW)
        if mode == "general":
            im["maskT"] = maskT
        in_maps.append(im)
    return mode, (bias_k, bias_v, bias_o), in_maps


def assemble(results, bo=None):
    """Sum per-core partial outputs into the full [B, S, D] output."""
    full = np.zeros((B, S, D), dtype=np.float32)
    for b in range(B):
        acc = results[4 * b]["out"].astype(np.float32)
        for c in range(4 * b + 1, 4 * b + 4):
            acc = acc + results[c]["out"].astype(np.float32)
        if bo is not None:
            acc = acc + bo
        full[b] = acc
    return full


def kernel(q, k, v, mask, Wk, bk, Wv, bv, Wo, bo):
    mode, (bias_k, bias_v, bias_o), in_maps = make_in_maps(
        q, k, v, mask, Wk, bk, Wv, bv, Wo, bo)
    nc = _get_nc(mode, bias_k, bias_v)
    res = bass_utils.run_bass_kernel_spmd(nc, in_maps, core_ids=list(range(NCORES)))
    bo_arr = np.asarray(bo, dtype=np.float32).reshape(-1) if bias_o else None
    return assemble(res.results, bo_arr)


# revision 33
# speedup vs baseline: 1.0695x; 1.0118x over previous
"""Multi-head attention (B=2, S=2048, D=1024, H=16) on 8 Trainium2 cores.

Sharding: data-parallel over the 2 batches x tensor-parallel over 4 groups
of 4 heads.  Core c handles batch c//4 and heads [4*(c%4) : 4*(c%4)+4]
(columns [256*(c%4) : +256] of Wk/Wv, same rows of Wo).  Each core produces
a partial [S, D] output (its heads' contribution to o @ Wo); the host sums
the 4 partials per batch (and adds bo once).

Per-core dataflow (bf16 operands cast on HOST, fp32 PSUM accumulation):
  qT,kT,vT [D,S] bf16 (host-pre-transposed + cast) load over fast HWDGE.
  Projections produce QT,KT [128,2,S] (head-major rows) and V [sk,hd] with
  an extra ones column.  Attention per head in "scores-transposed" layout
  [sk_part, sq_free]: scoresT = KT_j^T @ QT; the causal diagonal adds a
  bf16 -480 lower-triangular tile into PSUM via an identity matmul; exp on
  ScalarE (scale folded in; no max subtraction - scores are O(6));
  UT[65, S] += Vaug_j^T @ expT accumulated in PSUM, row 64 = softmax
  denominators (from the ones column).  Normalization is region-wise
  (512 cols at a time, as soon as that region's last k-block lands):
  u copy out of PSUM (DVE) -> sums row SBUF-DMA to partition 0 -> in-place
  DVE reciprocal [1,512] -> gpsimd partition_broadcast -> gpsimd multiply
  into oT [d_part, sq] (keeps the DVE queue free for PSUM evacuations).
  Final: out = oT^T @ Wo per 128-row block, bf16 DMA to HBM (host sums
  partials in fp32).
"""

import os
from contextlib import ExitStack

import numpy as np

import concourse.bass as bass
import concourse.tile as tile
from concourse import bacc, bass_utils, mybir
from concourse.masks import make_identity

B, S, D, H = 2, 2048, 1024, 16
HD = D // H            # 64
NCORES = 8
HPC = 4                # heads per core
CW = HPC * HD          # 256 weight cols per core
NCH = 4                # sequence chunks of 512
MASKVAL = -480.0       # additive pre-scale causal mask value (exp -> ~e-60)
S_INV = float(1.0 / (np.sqrt(np.float32(HD)) + np.float32(1e-8)))

F32 = mybir.dt.float32
BF16 = mybir.dt.bfloat16


def _build(mode: str, bias_k: bool, bias_v: bool):
    """Build + compile the SPMD program.

    mode: 'causal' | 'none' | 'general'
    """
    nc = bacc.Bacc("TRN2", target_bir_lowering=False, debug=False,
                   num_devices=NCORES)

    qT_d = nc.dram_tensor("qT", [D, S], BF16, kind="ExternalInput").ap()
    kT_d = nc.dram_tensor("kT", [D, S], BF16, kind="ExternalInput").ap()
    vT_d = nc.dram_tensor("vT", [D, S], BF16, kind="ExternalInput").ap()
    # weights are host-prepacked partition-major so each load is one
    # contiguous 4KB-per-partition DMA (512B chunks are ~3x slower)
    wk_d = nc.dram_tensor("wk", [128, 8, CW], BF16, kind="ExternalInput").ap()
    wv_d = nc.dram_tensor("wv", [128, 8, CW], BF16, kind="ExternalInput").ap()
    wo_d = nc.dram_tensor("wo", [128, 2, D], BF16, kind="ExternalInput").ap()
    bk_d = nc.dram_tensor("bk", [1, CW], BF16, kind="ExternalInput").ap() if bias_k else None
    bv_d = nc.dram_tensor("bv", [1, CW], BF16, kind="ExternalInput").ap() if bias_v else None
    maskT_d = (nc.dram_tensor("maskT", [S, S], BF16, kind="ExternalInput").ap()
               if mode == "general" else None)
    ones1_d = (nc.dram_tensor("ones1", [1, 512], BF16, kind="ExternalInput").ap()
               if (bias_k or bias_v) else None)
    out_d = nc.dram_tensor("out", [S, D], BF16, kind="ExternalOutput").ap()

    with tile.TileContext(nc) as tc, ExitStack() as ctx:
        sb1 = ctx.enter_context(tc.tile_pool(name="persist", bufs=1))
        qt_pool = ctx.enter_context(tc.tile_pool(name="qt", bufs=NCH))
        kt_pool = ctx.enter_context(tc.tile_pool(name="kt", bufs=NCH))
        v_pool = ctx.enter_context(tc.tile_pool(name="v", bufs=NCH))
        stage_pool = ctx.enter_context(tc.tile_pool(name="stage", bufs=12))
        exp_pool = ctx.enter_context(tc.tile_pool(name="exp", bufs=6))
        u_pool = ctx.enter_context(tc.tile_pool(name="u", bufs=4))
        srt_pool = ctx.enter_context(tc.tile_pool(name="srt", bufs=4))
        rcb_pool = ctx.enter_context(tc.tile_pool(name="rcb", bufs=4))
        bc_pool = ctx.enter_context(tc.tile_pool(name="bc", bufs=5))
        ottmp_pool = ctx.enter_context(tc.tile_pool(name="ottmp", bufs=2))
        outsb_pool = ctx.enter_context(tc.tile_pool(name="outsb", bufs=4))
        sc_pool = ctx.enter_context(tc.tile_pool(name="sc", bufs=3, space="PSUM"))
        ut_pool = ctx.enter_context(tc.tile_pool(name="ut", bufs=1, space="PSUM"))
        if mode == "general":
            mask_pool = ctx.enter_context(tc.tile_pool(name="mask", bufs=3))

        # ---- constants / weights (all bf16, fast HWDGE loads) --------
        wk_sb = sb1.tile([128, 8, CW], BF16)
        nc.sync.dma_start(wk_sb[:], wk_d[:])
        wv_sb = sb1.tile([128, 8, CW], BF16)
        wo_sb = sb1.tile([128, 2, D], BF16)
        if bias_k:
            bk_sb = sb1.tile([1, CW], BF16)
            nc.sync.dma_start(bk_sb[:], bk_d[:])
        if bias_v:
            bv_sb = sb1.tile([1, CW], BF16)
            nc.sync.dma_start(bv_sb[:], bv_d[:])
        if bias_k or bias_v:
            ones_sb = sb1.tile([1, 512], BF16)
            nc.sync.dma_start(ones_sb[:], ones1_d[:])
        if mode != "none":
            ident = sb1.tile([128, 128], BF16)
            make_identity(nc, ident[:])
        if mode == "causal":
            # dmask[p, f] = MASKVAL where f < p (sq < sk), else 0
            dmask = sb1.tile([128, 128], BF16)
            nc.gpsimd.memset(dmask[:], 0.0)
            nc.gpsimd.affine_select(
                out=dmask[:], in_=dmask[:],
                compare_op=mybir.AluOpType.is_ge,
                fill=MASKVAL, base=0,
                pattern=[[1, 128]], channel_multiplier=-1,
            )

        # V tiles: [128 sk, 4 blk, 4 head, 66] - col 64 is the ones column
        v_tiles = [v_pool.tile([128, 4, HPC, 66], BF16, tag="v", name=f"v{c}")
                   for c in range(NCH)]
        for c in range(NCH):
            nc.gpsimd.memset(v_tiles[c][:, :, :, 64:65], 1.0)
        qt_tiles = [qt_pool.tile([128, 2, 512], BF16, tag="qt", name=f"qt{c}")
                    for c in range(NCH)]
        kt_tiles = [kt_pool.tile([128, 2, 512], BF16, tag="kt", name=f"kt{c}")
                    for c in range(NCH)]
        oT_sb = sb1.tile([128, 2, S], BF16)

        def ps_copy(dst, src):
            nc.vector.tensor_copy(dst, src)

        # ---- phase 1: projections (helpers) --------------------------
        def emit_one_load(nm, td, c, hh, eng=None):
            sl = bass.ds(c * 512, 512)
            stg = stage_pool.tile([128, 4, 512], BF16, tag="stage",
                                  name=f"{nm}st{c}_{hh}")
            (eng or nc.sync).dma_start(
                stg[:], td.rearrange("(cc p) s -> p cc s", p=128)
                [:, bass.ds(4 * hh, 4), sl])
            return stg

        def emit_proj_loads(c):
            # load order = first-consumer order: c<2 units run K,Q then V
            order = ((("q", qT_d), ("k", kT_d), ("v", vT_d)) if c >= 2 else
                     (("k", kT_d), ("q", qT_d), ("v", vT_d)))
            ld = {nm: [emit_one_load(nm, td, c, hh) for hh in range(2)]
                  for nm, td in order}
            return ([ld["q"], ld["k"], ld["v"]] if c >= 2 else
                    [ld["k"], ld["v"], ld["q"]])

        def proj_mm_units(c, stages):
            """Yield once per PSUM accumulation group (small PE work unit)."""
            if c >= 2:
                (qst2, kst2, vst2) = stages
            else:
                (kst2, vst2, qst2) = stages
            class _Pair:
                def __init__(self, halves):
                    self.h = halves
                def __getitem__(self, key):
                    p, dc, rest = key[0], key[1], key[2:]
                    return self.h[dc // 4][(p, dc % 4) + rest]
            kst, vst, qst = _Pair(kst2), _Pair(vst2), _Pair(qst2)
            # KT / QT projections (transposed layout, 2 m-halves of 128)
            kq_order = (((qst, qt_tiles[c]), (kst, kt_tiles[c])) if c >= 2 else
                        ((kst, kt_tiles[c]), (qst, qt_tiles[c])))
            for ti, (st, dst) in enumerate(kq_order):
                for m in range(2):
                    ps = sc_pool.tile([128, 512], F32, tag="sc", name=f"psp{c}_{ti}_{m}")
                    first = True
                    if bias_k:
                        nc.tensor.matmul(ps[:], bk_sb[0:1, bass.ds(m * 128, 128)],
                                         ones_sb[0:1, :], start=True, stop=False)
                        first = False
                    for dc in range(8):
                        nc.tensor.matmul(
                            ps[:],
                            wk_sb[:, dc, bass.ds(m * 128, 128)],
                            st[:, dc, :],
                            start=first, stop=(dc == 7))
                        first = False
                    ps_copy(dst[:, m, :], ps[:])
                    yield
            # V projection (natural layout)
            for half in range(2):
                psv = sc_pool.tile([128, 512], F32, tag="sc", name=f"psv{c}_{half}")
                for loc in range(2):
                    blk = 2 * half + loc
                    reg = psv[:, bass.ds(loc * 256, 256)]
                    first = True
                    if bias_v:
                        nc.tensor.matmul(reg, ones_sb[0:1, 0:128], bv_sb[0:1, :],
                                         start=True, stop=False)
                        first = False
                    for dc in range(8):
                        nc.tensor.matmul(
                            reg,
                            vst[:, dc, bass.ds(blk * 128, 128)],
                            wv_sb[:, dc, :],
                            start=first, stop=(dc == 7))
                        first = False
                ps_copy(v_tiles[c][:, bass.ds(2 * half, 2), :, 0:64],
                        psv[:].rearrange("p (b h e) -> p b h e", b=2, h=HPC))
                yield

        # ---- phase 2: attention, one (head, sq-half) pass ------------
        full_grid = mode != "causal"

        # Normalization multiplies are deferred by one attention step so
        # the (in-order) DVE queue never sits waiting on the gpsimd
        # broadcast while a PE-critical PSUM evacuation is queued behind.
        deferred = []

        def flush_deferred():
            for f in deferred:
                f()
            deferred.clear()

        def attn_half(hl, half):
            m = hl // 2
            p0 = 64 * (hl % 2)
            base = 1024 * half
            regions = (2 * half, 2 * half + 1)
            ut = ut_pool.tile([128, 1024], F32, tag="ut", name=f"ut{hl}_{half}")

            if full_grid:
                steps = list(range(16))
                last_j = {r: 15 for r in regions}
            else:
                steps = list(range(8 * half + 8))
                last_j = {r: 4 * r + 3 for r in regions}

            win_ps = {}
            win_exp = {}

            def active(j):
                """absolute start col of k-block j's active window portion."""
                return base if full_grid else max(128 * j, base)

            def emit_scores(j):
                ps = sc_pool.tile([128, 1024], F32, tag="sc",
                                  name=f"sc{hl}_{half}_{j}")
                win_ps[j] = ps
                a0 = active(j)
                if mode == "general":
                    mt = mask_pool.tile([128, 1024], BF16, tag="mask",
                                        name=f"mt{hl}_{half}_{j}")
                    nc.sync.dma_start(
                        mt[:, a0 - base:],
                        maskT_d[bass.ds(128 * j, 128), bass.ds(a0, base + 1024 - a0)])
                lhsT = kt_tiles[j // 4][p0:p0 + 64, m, bass.ds(128 * (j % 4), 128)]
                for s in range(2):
                    lo, hi = base + 512 * s, base + 512 * s + 512
                    if hi <= a0:
                        continue
                    nlo = max(lo, a0)
                    n = hi - nlo
                    reg = ps[:, bass.ds(nlo - base, n)]
                    rhs = qt_tiles[nlo // 512][p0:p0 + 64, m, bass.ds(nlo % 512, n)]
                    diag_here = (mode == "causal") and lo <= 128 * j < hi
                    mask_here = (mode == "general")
                    nc.tensor.matmul(reg, lhsT, rhs, start=True,
                                     stop=not (diag_here or mask_here))
                    if diag_here:
                        nc.tensor.matmul(ps[:, bass.ds(128 * j - base, 128)],
                                         ident[:], dmask[:], start=False, stop=True)
                    elif mask_here:
                        nc.tensor.matmul(reg, ident[:],
                                         mt[:, bass.ds(nlo - base, n)],
                                         start=False, stop=True)

            def emit_exp(j):
                ps = win_ps[j]
                off = active(j) - base
                et = exp_pool.tile([128, 1024], BF16, tag="exp",
                                   name=f"e{hl}_{half}_{j}")
                win_exp[j] = et
                nc.scalar.activation(et[:, off:1024], ps[:, off:1024],
                                     mybir.ActivationFunctionType.Exp, scale=S_INV)

            def emit_pv(j):
                et = win_exp.pop(j)
                win_ps.pop(j)
                a0 = active(j)
                for s in range(2):
                    lo, hi = base + 512 * s, base + 512 * s + 512
                    if hi <= a0:
                        continue
                    nlo = max(lo, a0)
                    r = nlo // 512
                    nc.tensor.matmul(
                        ut[0:65, bass.ds(nlo - base, hi - nlo)],
                        v_tiles[j // 4][:, j % 4, hl, 0:65],
                        et[:, bass.ds(nlo - base, hi - nlo)],
                        start=(j == 0), stop=(j == last_j[r]))

            if p0 == 0:
                dst = oT_sb[0:64, m, bass.ds(base, 1024)]
                ott = None
            else:
                ott = ottmp_pool.tile([64, 1024], BF16, tag="ottmp",
                                      name=f"ott{hl}_{half}")
                dst = ott[:, :]

            def emit_norm(r):
                """copy U+sums out of PSUM, then recip -> bcast -> multiply.

                The DMA reshape to [128,4] keeps the reciprocal's per-lane
                free size at 4 (a [1,512] recip costs ~3.3us).  gpsimd runs
                ONLY partition_broadcast in steady state - mixing in other
                gpsimd op types causes ~7us DSP LIBRARY_RELOADs.  The DVE
                normalize-multiply is deferred one step (see above)."""
                u = u_pool.tile([65, 512], F32, tag="u", name=f"u{hl}_{r}")
                nc.vector.tensor_copy(u[:], ut[0:65, bass.ds(512 * r - base, 512)])
                srt = srt_pool.tile([128, 4], F32, tag="srt", name=f"srt{hl}_{r}")
                nc.sync.dma_start(srt[:], u[64:65, :])
                nc.vector.reciprocal(srt[:], srt[:])
                rcb = rcb_pool.tile([1, 512], F32, tag="rcb", name=f"rcb{hl}_{r}")
                nc.sync.dma_start(rcb[0:1, :], srt[:])
                bc = bc_pool.tile([64, 512], F32, tag="bc", name=f"bc{hl}_{r}")
                nc.gpsimd.partition_broadcast(bc[:], rcb[:], channels=64)

                def _mul():
                    nc.vector.tensor_mul(
                        dst[:, bass.ds(512 * r - base, 512)],
                        u[0:64, :],
                        bc[:, :])
                    if p0:
                        nc.sync.dma_start(
                            oT_sb[64:128, m, bass.ds(512 * r, 512)],
                            ott[:, bass.ds(512 * r - base, 512)])
                deferred.append(_mul)

            LOOKAHEAD = 2
            for i in range(min(LOOKAHEAD, len(steps))):
                emit_scores(steps[i])
            for i, j in enumerate(steps):
                flush_deferred()
                if i + LOOKAHEAD < len(steps):
                    emit_scores(steps[i + LOOKAHEAD])
                emit_exp(j)
                emit_pv(j)
                for r in regions:
                    if j == last_j[r]:
                        emit_norm(r)
                yield

        def emit_final(sb):
            # output stores go out on gpsimd/SWDGE so the sync queue stays
            # clear for the latency-critical softmax-norm reshape DMAs
            ob = outsb_pool.tile([128, D], BF16, tag="outsb", name=f"ob{sb}")
            for nh in range(2):
                ps = sc_pool.tile([128, 512], F32, tag="sc", name=f"pso{sb}_{nh}")
                for mm_ in range(2):
                    nc.tensor.matmul(
                        ps[:],
                        oT_sb[:, mm_, bass.ds(sb * 128, 128)],
                        wo_sb[:, mm_, bass.ds(nh * 512, 512)],
                        start=(mm_ == 0), stop=(mm_ == 1))
                ps_copy(ob[:, bass.ds(nh * 512, 512)], ps[:])
                nc.gpsimd.dma_start(
                    out_d[bass.ds(sb * 128, 128), bass.ds(nh * 512, 512)],
                    ob[:, bass.ds(nh * 512, 512)])

        def drain(gen):
            for _ in gen:
                pass

        def weave(step_gen, unit_gen, steps_per_unit):
            """Emit attention steps, inserting one PE-heavy unit every N."""
            i = 0
            for _ in step_gen:
                i += 1
                if i % steps_per_unit == 0:
                    next(unit_gen, None)
            for _ in unit_gen:
                pass

        def chain(*gens):
            for g in gens:
                yield from g

        # ---- orchestration: overlap proj DMA with attention ----------
        # load order matches first-consumer order: wk -> k0 -> q0 (KQ proj
        # units) -> wv -> v0 (V proj unit) -> chunk 1 -> wo
        kst0 = [emit_one_load("k", kT_d, 0, hh) for hh in range(2)]
        qst0 = [emit_one_load("q", qT_d, 0, hh) for hh in range(2)]
        nc.sync.dma_start(wv_sb[:], wv_d[:])
        vst0 = [emit_one_load("v", vT_d, 0, hh) for hh in range(2)]
        st0 = [kst0, vst0, qst0]
        st1 = emit_proj_loads(1)
        nc.sync.dma_start(wo_sb[:], wo_d[:])
        drain(proj_mm_units(0, st0))
        st2 = emit_proj_loads(2)
        st3 = emit_proj_loads(3)
        drain(proj_mm_units(1, st1))
        if full_grid:
            # every k-block needs every chunk: project everything first
            drain(proj_mm_units(2, st2))
            drain(proj_mm_units(3, st3))
            for hl in (1, 3, 0, 2):
                drain(attn_half(hl, 0))
        else:
            half0s = chain(*[attn_half(hl, 0) for hl in (1, 3, 0, 2)])
            proj23 = chain(proj_mm_units(2, st2), proj_mm_units(3, st3))
            # small un-woven prefix: let the c2/c3 stage DMAs land so an
            # early proj unit can't head-of-line-block the in-order PE.
            for _ in range(6):
                next(half0s, None)
            weave(half0s, proj23, 1)

        def final_units(lo, hi):
            for sb in range(lo, hi):
                emit_final(sb)
                yield

        half1s_012 = chain(*[attn_half(hl, 1) for hl in (1, 3, 0)])
        weave(half1s_012, final_units(0, 8), 5)
        # finals 8-11 run AFTER the last pass: woven mid-pass they would
        # head-of-line-stall the PE on the region-2 norm chain, delaying
        # the last steps and with them the whole region-3 endgame chain.
        drain(attn_half(2, 1))
        for _ in final_units(8, 12):
            pass
        # endgame: the last pass's region-3 norm chain has ~7us of
        # cross-engine latency.  The m=0 halves of the last finals only
        # need heads 0/1 (normalized long ago), so open those accumulation
        # groups now and let the PE chew on them while the chain drains.
        def open_final(sb):
            ps = sc_pool.tile([128, 1024], F32, tag="sc", name=f"psoX{sb}")
            for nh in range(2):
                nc.tensor.matmul(
                    ps[:, bass.ds(nh * 512, 512)],
                    oT_sb[:, 0, bass.ds(sb * 128, 128)],
                    wo_sb[:, 0, bass.ds(nh * 512, 512)],
                    start=True, stop=False)
            return ps

        def close_final(sb, ps):
            ob = outsb_pool.tile([128, D], BF16, tag="outsb", name=f"ob{sb}")
            for nh in range(2):
                nc.tensor.matmul(
                    ps[:, bass.ds(nh * 512, 512)],
                    oT_sb[:, 1, bass.ds(sb * 128, 128)],
                    wo_sb[:, 1, bass.ds(nh * 512, 512)],
                    start=False, stop=True)
                ps_copy(ob[:, bass.ds(nh * 512, 512)],
                        ps[:, bass.ds(nh * 512, 512)])
                nc.gpsimd.dma_start(
                    out_d[bass.ds(sb * 128, 128), bass.ds(nh * 512, 512)],
                    ob[:, bass.ds(nh * 512, 512)])

        ps12 = open_final(12)
        ps13 = open_final(13)
        flush_deferred()
        close_final(12, ps12)
        close_final(13, ps13)
        for sb in range(14, 16):
            emit_final(sb)


    nc.compile()
    return nc


_CACHE = {}


def _get_nc(mode, bias_k, bias_v):
    key = (mode, bias_k, bias_v)
    if key not in _CACHE:
        _CACHE[key] = _build(mode, bias_k, bias_v)
    return _CACHE[key]


def make_in_maps(q, k, v, mask, Wk, bk, Wv, bv, Wo, bo):
    """Host-side sharding. Returns (mode, bias flags, in_maps)."""
    import ml_dtypes

    BF = ml_dtypes.bfloat16

    q = np.asarray(q, dtype=np.float32)
    k = np.asarray(k, dtype=np.float32)
    v = np.asarray(v, dtype=np.float32)
    Wk = np.asarray(Wk, dtype=np.float32).astype(BF)
    Wv = np.asarray(Wv, dtype=np.float32).astype(BF)
    Wo = np.asarray(Wo, dtype=np.float32).astype(BF)
    bk = np.asarray(bk, dtype=np.float32).reshape(-1)
    bv = np.asarray(bv, dtype=np.float32).reshape(-1)
    bo = np.asarray(bo, dtype=np.float32).reshape(-1)
    mask2d = np.asarray(mask, dtype=np.float32).reshape(S, S)

    if not mask2d.any():
        mode = "none"
    elif np.array_equal(mask2d, np.triu(np.ones((S, S), np.float32), 1)):
        mode = "causal"
    else:
        mode = "general"
    bias_k, bias_v, bias_o = bool(bk.any()), bool(bv.any()), bool(bo.any())

    qT = [np.ascontiguousarray(q[b].T).astype(BF) for b in range(B)]
    kT = [np.ascontiguousarray(k[b].T).astype(BF) for b in range(B)]
    vT = [np.ascontiguousarray(v[b].T).astype(BF) for b in range(B)]
    if mode == "general":
        # pre-scale so adding before the fused exp scale matches the
        # reference's post-scale add:  (raw + m)*S_INV == raw*S_INV + mask*(-1e9)
        maskT = np.ascontiguousarray(
            (mask2d.T * np.float32(-1e9 / S_INV)).astype(BF))

    def pack_pmajor(w, groups):
        """[groups*128, n] -> [128, groups, n] partition-major prepack."""
        n = w.shape[1]
        return np.ascontiguousarray(
            w.reshape(groups, 128, n).transpose(1, 0, 2))

    in_maps = []
    for core in range(NCORES):
        b, g = divmod(core, HPC)
        cs = slice(CW * g, CW * (g + 1))
        im = {
            "qT": qT[b], "kT": kT[b], "vT": vT[b],
            "wk": pack_pmajor(np.ascontiguousarray(Wk[:, cs]), 8),
            "wv": pack_pmajor(np.ascontiguousarray(Wv[:, cs]), 8),
            "wo": pack_pmajor(np.ascontiguousarray(Wo[cs, :]), 2),
        }
        if bias_k or bias_v:
            im["ones1"] = np.ones((1, 512), dtype=BF)
        if bias_k:
            im["bk"] = np.ascontiguousarray(bk[cs].astype(BF)).reshape(1, CW)
        if bias_v:
            im["bv"] = np.ascontiguousarray(bv[cs].astype(BF)).reshape(1, CW)
        if mode == "general":
            im["maskT"] = maskT
        in_maps.append(im)
    return mode, (bias_k, bias_v, bias_o), in_maps


def assemble(results, bo=None):
    """Sum per-core partial outputs into the full [B, S, D] output."""
    full = np.zeros((B, S, D), dtype=np.float32)
    for b in range(B):
        acc = results[4 * b]["out"].astype(np.float32)
        for c in range(4 * b + 1, 4 * b + 4):
            acc = acc + results[c]["out"].astype(np.float32)
        if bo is not None:
            acc = acc + bo
        full[b] = acc
    return full


def kernel(q, k, v, mask, Wk, bk, Wv, bv, Wo, bo):
    mode, (bias_k, bias_v, bias_o), in_maps = make_in_maps(
        q, k, v, mask, Wk, bk, Wv, bv, Wo, bo)
    nc = _get_nc(mode, bias_k, bias_v)
    res = bass_utils.run_bass_kernel_spmd(nc, in_maps, core_ids=list(range(NCORES)))
    bo_arr = np.asarray(bo, dtype=np.float32).reshape(-1) if bias_o else None
    return assemble(res.results, bo_arr)


# revision 36
# speedup vs baseline: 1.0710x; 1.0014x over previous
"""Multi-head attention (B=2, S=2048, D=1024, H=16) on 8 Trainium2 cores.

Sharding: data-parallel over the 2 batches x tensor-parallel over 4 groups
of 4 heads.  Core c handles batch c//4 and heads [4*(c%4) : 4*(c%4)+4]
(columns [256*(c%4) : +256] of Wk/Wv, same rows of Wo).  Each core produces
a partial [S, D] output (its heads' contribution to o @ Wo); the host sums
the 4 partials per batch (and adds bo once).

Per-core dataflow (bf16 operands cast on HOST, fp32 PSUM accumulation):
  qT,kT,vT [D,S] bf16 (host-pre-transposed + cast) load over fast HWDGE.
  Projections produce QT,KT [128,2,S] (head-major rows) and V [sk,hd] with
  an extra ones column.  Attention per head in "scores-transposed" layout
  [sk_part, sq_free]: scoresT = KT_j^T @ QT; the causal diagonal adds a
  bf16 -480 lower-triangular tile into PSUM via an identity matmul; exp on
  ScalarE (scale folded in; no max subtraction - scores are O(6));
  UT[65, S] += Vaug_j^T @ expT accumulated in PSUM, row 64 = softmax
  denominators (from the ones column).  Normalization is region-wise
  (512 cols at a time, as soon as that region's last k-block lands):
  u copy out of PSUM (DVE) -> sums row SBUF-DMA to partition 0 -> in-place
  DVE reciprocal [1,512] -> gpsimd partition_broadcast -> gpsimd multiply
  into oT [d_part, sq] (keeps the DVE queue free for PSUM evacuations).
  Final: out = oT^T @ Wo per 128-row block, bf16 DMA to HBM (host sums
  partials in fp32).
"""

import os
from contextlib import ExitStack

import numpy as np

import concourse.bass as bass
import concourse.tile as tile
from concourse import bacc, bass_utils, mybir
from concourse.masks import make_identity

B, S, D, H = 2, 2048, 1024, 16
HD = D // H            # 64
NCORES = 8
HPC = 4                # heads per core
CW = HPC * HD          # 256 weight cols per core
NCH = 4                # sequence chunks of 512
MASKVAL = -480.0       # additive pre-scale causal mask value (exp -> ~e-60)
S_INV = float(1.0 / (np.sqrt(np.float32(HD)) + np.float32(1e-8)))

F32 = mybir.dt.float32
BF16 = mybir.dt.bfloat16


def _build(mode: str, bias_k: bool, bias_v: bool):
    """Build + compile the SPMD program.

    mode: 'causal' | 'none' | 'general'
    """
    nc = bacc.Bacc("TRN2", target_bir_lowering=False, debug=False,
                   num_devices=NCORES)

    qT_d = nc.dram_tensor("qT", [D, S], BF16, kind="ExternalInput").ap()
    kT_d = nc.dram_tensor("kT", [D, S], BF16, kind="ExternalInput").ap()
    vT_d = nc.dram_tensor("vT", [D, S], BF16, kind="ExternalInput").ap()
    # weights are host-prepacked partition-major so each load is one
    # contiguous 4KB-per-partition DMA (512B chunks are ~3x slower)
    wk_d = nc.dram_tensor("wk", [128, 8, CW], BF16, kind="ExternalInput").ap()
    wv_d = nc.dram_tensor("wv", [128, 8, CW], BF16, kind="ExternalInput").ap()
    wo_d = nc.dram_tensor("wo", [128, 2, D], BF16, kind="ExternalInput").ap()
    bk_d = nc.dram_tensor("bk", [1, CW], BF16, kind="ExternalInput").ap() if bias_k else None
    bv_d = nc.dram_tensor("bv", [1, CW], BF16, kind="ExternalInput").ap() if bias_v else None
    maskT_d = (nc.dram_tensor("maskT", [S, S], BF16, kind="ExternalInput").ap()
               if mode == "general" else None)
    ones1_d = (nc.dram_tensor("ones1", [1, 512], BF16, kind="ExternalInput").ap()
               if (bias_k or bias_v) else None)
    out_d = nc.dram_tensor("out", [S, D], BF16, kind="ExternalOutput").ap()

    with tile.TileContext(nc) as tc, ExitStack() as ctx:
        sb1 = ctx.enter_context(tc.tile_pool(name="persist", bufs=1))
        qt_pool = ctx.enter_context(tc.tile_pool(name="qt", bufs=NCH))
        kt_pool = ctx.enter_context(tc.tile_pool(name="kt", bufs=NCH))
        v_pool = ctx.enter_context(tc.tile_pool(name="v", bufs=NCH))
        stage_pool = ctx.enter_context(tc.tile_pool(name="stage", bufs=12))
        exp_pool = ctx.enter_context(tc.tile_pool(name="exp", bufs=6))
        u_pool = ctx.enter_context(tc.tile_pool(name="u", bufs=4))
        srt_pool = ctx.enter_context(tc.tile_pool(name="srt", bufs=4))
        rcb_pool = ctx.enter_context(tc.tile_pool(name="rcb", bufs=4))
        bc_pool = ctx.enter_context(tc.tile_pool(name="bc", bufs=5))
        ottmp_pool = ctx.enter_context(tc.tile_pool(name="ottmp", bufs=2))
        outsb_pool = ctx.enter_context(tc.tile_pool(name="outsb", bufs=4))
        sc_pool = ctx.enter_context(tc.tile_pool(name="sc", bufs=3, space="PSUM"))
        ut_pool = ctx.enter_context(tc.tile_pool(name="ut", bufs=1, space="PSUM"))
        if mode == "general":
            mask_pool = ctx.enter_context(tc.tile_pool(name="mask", bufs=3))

        # ---- constants / weights (all bf16, fast HWDGE loads) --------
        wk_sb = sb1.tile([128, 8, CW], BF16)
        nc.sync.dma_start(wk_sb[:], wk_d[:])
        wv_sb = sb1.tile([128, 8, CW], BF16)
        wo_sb = sb1.tile([128, 2, D], BF16)
        if bias_k:
            bk_sb = sb1.tile([1, CW], BF16)
            nc.sync.dma_start(bk_sb[:], bk_d[:])
        if bias_v:
            bv_sb = sb1.tile([1, CW], BF16)
            nc.sync.dma_start(bv_sb[:], bv_d[:])
        if bias_k or bias_v:
            ones_sb = sb1.tile([1, 512], BF16)
            nc.sync.dma_start(ones_sb[:], ones1_d[:])
        if mode != "none":
            ident = sb1.tile([128, 128], BF16)
            make_identity(nc, ident[:])
        if mode == "causal":
            # dmask[p, f] = MASKVAL where f < p (sq < sk), else 0
            dmask = sb1.tile([128, 128], BF16)
            nc.gpsimd.memset(dmask[:], 0.0)
            nc.gpsimd.affine_select(
                out=dmask[:], in_=dmask[:],
                compare_op=mybir.AluOpType.is_ge,
                fill=MASKVAL, base=0,
                pattern=[[1, 128]], channel_multiplier=-1,
            )

        # V tiles: [128 sk, 4 blk, 4 head, 66] - col 64 is the ones column
        v_tiles = [v_pool.tile([128, 4, HPC, 66], BF16, tag="v", name=f"v{c}")
                   for c in range(NCH)]
        for c in range(NCH):
            nc.gpsimd.memset(v_tiles[c][:, :, :, 64:65], 1.0)
        qt_tiles = [qt_pool.tile([128, 2, 512], BF16, tag="qt", name=f"qt{c}")
                    for c in range(NCH)]
        kt_tiles = [kt_pool.tile([128, 2, 512], BF16, tag="kt", name=f"kt{c}")
                    for c in range(NCH)]
        oT_sb = sb1.tile([128, 2, S], BF16)

        def ps_copy(dst, src):
            nc.vector.tensor_copy(dst, src)

        # ---- phase 1: projections (helpers) --------------------------
        def emit_one_load(nm, td, c, hh, eng=None):
            sl = bass.ds(c * 512, 512)
            stg = stage_pool.tile([128, 4, 512], BF16, tag="stage",
                                  name=f"{nm}st{c}_{hh}")
            (eng or nc.sync).dma_start(
                stg[:], td.rearrange("(cc p) s -> p cc s", p=128)
                [:, bass.ds(4 * hh, 4), sl])
            return stg

        def emit_proj_loads(c):
            # load order = first-consumer order: c<2 units run K,Q then V
            order = ((("q", qT_d), ("k", kT_d), ("v", vT_d)) if c >= 2 else
                     (("k", kT_d), ("q", qT_d), ("v", vT_d)))
            ld = {nm: [emit_one_load(nm, td, c, hh) for hh in range(2)]
                  for nm, td in order}
            return ([ld["q"], ld["k"], ld["v"]] if c >= 2 else
                    [ld["k"], ld["v"], ld["q"]])

        def proj_mm_units(c, stages):
            """Yield once per PSUM accumulation group (small PE work unit)."""
            if c >= 2:
                (qst2, kst2, vst2) = stages
            else:
                (kst2, vst2, qst2) = stages
            class _Pair:
                def __init__(self, halves):
                    self.h = halves
                def __getitem__(self, key):
                    p, dc, rest = key[0], key[1], key[2:]
                    return self.h[dc // 4][(p, dc % 4) + rest]
            kst, vst, qst = _Pair(kst2), _Pair(vst2), _Pair(qst2)
            # KT / QT projections (transposed layout, 2 m-halves of 128)
            kq_order = (((qst, qt_tiles[c]), (kst, kt_tiles[c])) if c >= 2 else
                        ((kst, kt_tiles[c]), (qst, qt_tiles[c])))
            for ti, (st, dst) in enumerate(kq_order):
                for m in range(2):
                    ps = sc_pool.tile([128, 512], F32, tag="sc", name=f"psp{c}_{ti}_{m}")
                    first = True
                    if bias_k:
                        nc.tensor.matmul(ps[:], bk_sb[0:1, bass.ds(m * 128, 128)],
                                         ones_sb[0:1, :], start=True, stop=False)
                        first = False
                    for dc in range(8):
                        nc.tensor.matmul(
                            ps[:],
                            wk_sb[:, dc, bass.ds(m * 128, 128)],
                            st[:, dc, :],
                            start=first, stop=(dc == 7))
                        first = False
                    ps_copy(dst[:, m, :], ps[:])
                    yield
            # V projection (natural layout)
            for half in range(2):
                psv = sc_pool.tile([128, 512], F32, tag="sc", name=f"psv{c}_{half}")
                for loc in range(2):
                    blk = 2 * half + loc
                    reg = psv[:, bass.ds(loc * 256, 256)]
                    first = True
                    if bias_v:
                        nc.tensor.matmul(reg, ones_sb[0:1, 0:128], bv_sb[0:1, :],
                                         start=True, stop=False)
                        first = False
                    for dc in range(8):
                        nc.tensor.matmul(
                            reg,
                            vst[:, dc, bass.ds(blk * 128, 128)],
                            wv_sb[:, dc, :],
                            start=first, stop=(dc == 7))
                        first = False
                ps_copy(v_tiles[c][:, bass.ds(2 * half, 2), :, 0:64],
                        psv[:].rearrange("p (b h e) -> p b h e", b=2, h=HPC))
                yield

        # ---- phase 2: attention, one (head, sq-half) pass ------------
        full_grid = mode != "causal"

        # Normalization multiplies are deferred by one attention step so
        # the (in-order) DVE queue never sits waiting on the gpsimd
        # broadcast while a PE-critical PSUM evacuation is queued behind.
        deferred = []

        def flush_deferred():
            for f in deferred:
                f()
            deferred.clear()

        def attn_half(hl, half):
            m = hl // 2
            p0 = 64 * (hl % 2)
            base = 1024 * half
            regions = (2 * half, 2 * half + 1)
            ut = ut_pool.tile([128, 1024], F32, tag="ut", name=f"ut{hl}_{half}")

            if full_grid:
                steps = list(range(16))
                last_j = {r: 15 for r in regions}
            else:
                steps = list(range(8 * half + 8))
                last_j = {r: 4 * r + 3 for r in regions}

            win_ps = {}
            win_exp = {}

            def active(j):
                """absolute start col of k-block j's active window portion."""
                return base if full_grid else max(128 * j, base)

            def emit_scores(j):
                ps = sc_pool.tile([128, 1024], F32, tag="sc",
                                  name=f"sc{hl}_{half}_{j}")
                win_ps[j] = ps
                a0 = active(j)
                if mode == "general":
                    mt = mask_pool.tile([128, 1024], BF16, tag="mask",
                                        name=f"mt{hl}_{half}_{j}")
                    nc.sync.dma_start(
                        mt[:, a0 - base:],
                        maskT_d[bass.ds(128 * j, 128), bass.ds(a0, base + 1024 - a0)])
                lhsT = kt_tiles[j // 4][p0:p0 + 64, m, bass.ds(128 * (j % 4), 128)]
                for s in range(2):
                    lo, hi = base + 512 * s, base + 512 * s + 512
                    if hi <= a0:
                        continue
                    nlo = max(lo, a0)
                    n = hi - nlo
                    reg = ps[:, bass.ds(nlo - base, n)]
                    rhs = qt_tiles[nlo // 512][p0:p0 + 64, m, bass.ds(nlo % 512, n)]
                    diag_here = (mode == "causal") and lo <= 128 * j < hi
                    mask_here = (mode == "general")
                    nc.tensor.matmul(reg, lhsT, rhs, start=True,
                                     stop=not (diag_here or mask_here))
                    if diag_here:
                        nc.tensor.matmul(ps[:, bass.ds(128 * j - base, 128)],
                                         ident[:], dmask[:], start=False, stop=True)
                    elif mask_here:
                        nc.tensor.matmul(reg, ident[:],
                                         mt[:, bass.ds(nlo - base, n)],
                                         start=False, stop=True)

            def emit_exp(j):
                ps = win_ps[j]
                off = active(j) - base
                et = exp_pool.tile([128, 1024], BF16, tag="exp",
                                   name=f"e{hl}_{half}_{j}")
                win_exp[j] = et
                nc.scalar.activation(et[:, off:1024], ps[:, off:1024],
                                     mybir.ActivationFunctionType.Exp, scale=S_INV)

            def emit_pv(j):
                et = win_exp.pop(j)
                win_ps.pop(j)
                a0 = active(j)
                for s in range(2):
                    lo, hi = base + 512 * s, base + 512 * s + 512
                    if hi <= a0:
                        continue
                    nlo = max(lo, a0)
                    r = nlo // 512
                    nc.tensor.matmul(
                        ut[0:65, bass.ds(nlo - base, hi - nlo)],
                        v_tiles[j // 4][:, j % 4, hl, 0:65],
                        et[:, bass.ds(nlo - base, hi - nlo)],
                        start=(j == 0), stop=(j == last_j[r]))

            if p0 == 0:
                dst = oT_sb[0:64, m, bass.ds(base, 1024)]
                ott = None
            else:
                ott = ottmp_pool.tile([64, 1024], BF16, tag="ottmp",
                                      name=f"ott{hl}_{half}")
                dst = ott[:, :]

            def emit_norm(r):
                """copy U+sums out of PSUM, then recip -> bcast -> multiply.

                The DMA reshape to [128,4] keeps the reciprocal's per-lane
                free size at 4 (a [1,512] recip costs ~3.3us).  gpsimd runs
                ONLY partition_broadcast in steady state - mixing in other
                gpsimd op types causes ~7us DSP LIBRARY_RELOADs.  The DVE
                normalize-multiply is deferred one step (see above)."""
                u = u_pool.tile([65, 512], F32, tag="u", name=f"u{hl}_{r}")
                nc.vector.tensor_copy(u[:], ut[0:65, bass.ds(512 * r - base, 512)])
                srt = srt_pool.tile([128, 4], F32, tag="srt", name=f"srt{hl}_{r}")
                nc.sync.dma_start(srt[:], u[64:65, :])
                nc.vector.reciprocal(srt[:], srt[:])
                rcb = rcb_pool.tile([1, 512], F32, tag="rcb", name=f"rcb{hl}_{r}")
                nc.sync.dma_start(rcb[0:1, :], srt[:])
                bc = bc_pool.tile([64, 512], F32, tag="bc", name=f"bc{hl}_{r}")
                nc.gpsimd.partition_broadcast(bc[:], rcb[:], channels=64)

                def _mul():
                    nc.vector.tensor_mul(
                        dst[:, bass.ds(512 * r - base, 512)],
                        u[0:64, :],
                        bc[:, :])
                    if p0:
                        nc.sync.dma_start(
                            oT_sb[64:128, m, bass.ds(512 * r, 512)],
                            ott[:, bass.ds(512 * r - base, 512)])
                deferred.append(_mul)

            LOOKAHEAD = 2
            for i in range(min(LOOKAHEAD, len(steps))):
                emit_scores(steps[i])
            for i, j in enumerate(steps):
                flush_deferred()
                if i + LOOKAHEAD < len(steps):
                    emit_scores(steps[i + LOOKAHEAD])
                emit_exp(j)
                emit_pv(j)
                for r in regions:
                    if j == last_j[r]:
                        emit_norm(r)
                yield

        def emit_final(sb, out_eng=None):
            # mid-kernel output stores go out on gpsimd/SWDGE so the sync
            # queue stays clear for the latency-critical softmax-norm
            # reshape DMAs; the tail finals use the (by then idle) sync
            # queue whose HWDGE acks faster
            ob = outsb_pool.tile([128, D], BF16, tag="outsb", name=f"ob{sb}")
            for nh in range(2):
                ps = sc_pool.tile([128, 512], F32, tag="sc", name=f"pso{sb}_{nh}")
                for mm_ in range(2):
                    nc.tensor.matmul(
                        ps[:],
                        oT_sb[:, mm_, bass.ds(sb * 128, 128)],
                        wo_sb[:, mm_, bass.ds(nh * 512, 512)],
                        start=(mm_ == 0), stop=(mm_ == 1))
                ps_copy(ob[:, bass.ds(nh * 512, 512)], ps[:])
                (out_eng or nc.gpsimd).dma_start(
                    out_d[bass.ds(sb * 128, 128), bass.ds(nh * 512, 512)],
                    ob[:, bass.ds(nh * 512, 512)])

        def drain(gen):
            for _ in gen:
                pass

        def weave(step_gen, unit_gen, steps_per_unit):
            """Emit attention steps, inserting one PE-heavy unit every N."""
            i = 0
            for _ in step_gen:
                i += 1
                if i % steps_per_unit == 0:
                    next(unit_gen, None)
            for _ in unit_gen:
                pass

        def chain(*gens):
            for g in gens:
                yield from g

        # ---- orchestration: overlap proj DMA with attention ----------
        # load order matches first-consumer order: wk -> k0 -> q0 (KQ proj
        # units) -> wv -> v0 (V proj unit) -> chunk 1 -> wo
        kst0 = [emit_one_load("k", kT_d, 0, hh) for hh in range(2)]
        qst0 = [emit_one_load("q", qT_d, 0, hh) for hh in range(2)]
        nc.sync.dma_start(wv_sb[:], wv_d[:])
        vst0 = [emit_one_load("v", vT_d, 0, hh) for hh in range(2)]
        st0 = [kst0, vst0, qst0]
        st1 = emit_proj_loads(1)
        nc.sync.dma_start(wo_sb[:], wo_d[:])
        drain(proj_mm_units(0, st0))
        st2 = emit_proj_loads(2)
        st3 = emit_proj_loads(3)
        drain(proj_mm_units(1, st1))
        if full_grid:
            # every k-block needs every chunk: project everything first
            drain(proj_mm_units(2, st2))
            drain(proj_mm_units(3, st3))
            for hl in (1, 3, 0, 2):
                drain(attn_half(hl, 0))
        else:
            half0s = chain(*[attn_half(hl, 0) for hl in (1, 3, 0, 2)])
            proj23 = chain(proj_mm_units(2, st2), proj_mm_units(3, st3))
            # small un-woven prefix: let the c2/c3 stage DMAs land so an
            # early proj unit can't head-of-line-block the in-order PE.
            for _ in range(6):
                next(half0s, None)
            weave(half0s, proj23, 1)

        def final_units(lo, hi):
            for sb in range(lo, hi):
                emit_final(sb)
                yield

        half1s_012 = chain(*[attn_half(hl, 1) for hl in (1, 3, 0)])
        weave(half1s_012, final_units(0, 8), 5)
        # finals 8-11 run AFTER the last pass: woven mid-pass they would
        # head-of-line-stall the PE on the region-2 norm chain, delaying
        # the last steps and with them the whole region-3 endgame chain.
        drain(attn_half(2, 1))
        for _ in final_units(8, 12):
            pass
        # endgame: the last pass's region-3 norm chain has ~7us of
        # cross-engine latency.  The m=0 halves of the last finals only
        # need heads 0/1 (normalized long ago), so open those accumulation
        # groups now and let the PE chew on them while the chain drains.
        def open_final(sb):
            ps = sc_pool.tile([128, 1024], F32, tag="sc", name=f"psoX{sb}")
            for nh in range(2):
                nc.tensor.matmul(
                    ps[:, bass.ds(nh * 512, 512)],
                    oT_sb[:, 0, bass.ds(sb * 128, 128)],
                    wo_sb[:, 0, bass.ds(nh * 512, 512)],
                    start=True, stop=False)
            return ps

        def close_final(sb, ps):
            ob = outsb_pool.tile([128, D], BF16, tag="outsb", name=f"ob{sb}")
            for nh in range(2):
                nc.tensor.matmul(
                    ps[:, bass.ds(nh * 512, 512)],
                    oT_sb[:, 1, bass.ds(sb * 128, 128)],
                    wo_sb[:, 1, bass.ds(nh * 512, 512)],
                    start=False, stop=True)
                ps_copy(ob[:, bass.ds(nh * 512, 512)],
                        ps[:, bass.ds(nh * 512, 512)])
                nc.sync.dma_start(
                    out_d[bass.ds(sb * 128, 128), bass.ds(nh * 512, 512)],
                    ob[:, bass.ds(nh * 512, 512)])

        ps12 = open_final(12)
        ps13 = open_final(13)
        flush_deferred()
        close_final(12, ps12)
        close_final(13, ps13)
        for sb in range(14, 16):
            emit_final(sb, out_eng=nc.sync)


    nc.compile()
    return nc


_CACHE = {}


def _get_nc(mode, bias_k, bias_v):
    key = (mode, bias_k, bias_v)
    if key not in _CACHE:
        _CACHE[key] = _build(mode, bias_k, bias_v)
    return _CACHE[key]


def make_in_maps(q, k, v, mask, Wk, bk, Wv, bv, Wo, bo):
    """Host-side sharding. Returns (mode, bias flags, in_maps)."""
    import ml_dtypes

    BF = ml_dtypes.bfloat16

    q = np.asarray(q, dtype=np.float32)
    k = np.asarray(k, dtype=np.float32)
    v = np.asarray(v, dtype=np.float32)
    Wk = np.asarray(Wk, dtype=np.float32).astype(BF)
    Wv = np.asarray(Wv, dtype=np.float32).astype(BF)
    Wo = np.asarray(Wo, dtype=np.float32).astype(BF)
    bk = np.asarray(bk, dtype=np.float32).reshape(-1)
    bv = np.asarray(bv, dtype=np.float32).reshape(-1)
    bo = np.asarray(bo, dtype=np.float32).reshape(-1)
    mask2d = np.asarray(mask, dtype=np.float32).reshape(S, S)

    if not mask2d.any():
        mode = "none"
    elif np.array_equal(mask2d, np.triu(np.ones((S, S), np.float32), 1)):
        mode = "causal"
    else:
        mode = "general"
    bias_k, bias_v, bias_o = bool(bk.any()), bool(bv.any()), bool(bo.any())

    qT = [np.ascontiguousarray(q[b].T).astype(BF) for b in range(B)]
    kT = [np.ascontiguousarray(k[b].T).astype(BF) for b in range(B)]
    vT = [np.ascontiguousarray(v[b].T).astype(BF) for b in range(B)]
    if mode == "general":
        # pre-scale so adding before the fused exp scale matches the
        # reference's post-scale add:  (raw + m)*S_INV == raw*S_INV + mask*(-1e9)
        maskT = np.ascontiguousarray(
            (mask2d.T * np.float32(-1e9 / S_INV)).astype(BF))

    def pack_pmajor(w, groups):
        """[groups*128, n] -> [128, groups, n] partition-major prepack."""
        n = w.shape[1]
        return np.ascontiguousarray(
            w.reshape(groups, 128, n).transpose(1, 0, 2))

    in_maps = []
    for core in range(NCORES):
        b, g = divmod(core, HPC)
        cs = slice(CW * g, CW * (g + 1))
        im = {
            "qT": qT[b], "kT": kT[b], "vT": vT[b],
            "wk": pack_pmajor(np.ascontiguousarray(Wk[:, cs]), 8),
            "wv": pack_pmajor(np.ascontiguousarray(Wv[:, cs]), 8),
            "wo": pack_pmajor(np.ascontiguousarray(Wo[cs, :]), 2),
        }
        if bias_k or bias_v:
            im["ones1"] = np.ones((1, 512), dtype=BF)
        if bias_k:
            im["bk"] = np.ascontiguousarray(bk[cs].astype(BF)).reshape(1, CW)
        if bias_v:
            im["bv"] = np.ascontiguousarray(bv[cs].astype(BF)).reshape(1, CW)
        if mode == "general":
            im["maskT"] = maskT
        in_maps.append(im)
    return mode, (bias_k, bias_v, bias_o), in_maps


def assemble(results, bo=None):
    """Sum per-core partial outputs into the full [B, S, D] output."""
    full = np.zeros((B, S, D), dtype=np.float32)
    for b in range(B):
        acc = results[4 * b]["out"].astype(np.float32)
        for c in range(4 * b + 1, 4 * b + 4):
            acc = acc + results[c]["out"].astype(np.float32)
        if bo is not None:
            acc = acc + bo
        full[b] = acc
    return full


def kernel(q, k, v, mask, Wk, bk, Wv, bv, Wo, bo):
    mode, (bias_k, bias_v, bias_o), in_maps = make_in_maps(
        q, k, v, mask, Wk, bk, Wv, bv, Wo, bo)
    nc = _get_nc(mode, bias_k, bias_v)
    res = bass_utils.run_bass_kernel_spmd(nc, in_maps, core_ids=list(range(NCORES)))
    bo_arr = np.asarray(bo, dtype=np.float32).reshape(-1) if bias_o else None
    return assemble(res.results, bo_arr)
